# revision 2
# baseline (speedup 1.0000x reference)
"""GAT-style attention (gnn_message_passing) Trainium2 kernel, 8-core row-parallel.

Math (algebraically identical to the reference masked-softmax attention):
  E = relu(h @ P)                 [N,3]
  W' = max(exp(E - 4ln2), 1/16)   (= exp(relu(E))/16, fp16-safe range)
  denom'[i,k] = sum_j A[i,j] W'[j,k]   (k=3 slot sums ones -> rowsum[i])
  R'[i,k] = rowsum[i] / denom'[i,k]
  ct[j,i]  = sum_k W'[j,k] R'[i,k] = rowsum[i] * C[i,j]
  mt[j,i]  = A[i,j] * ct[j,i]
  out[i,:] = sum_j mt[j,i] h[j,:]

Two SPMD programs (cost-modeled collectives are ~15us fixed -> too slow; the
tiny [4096,3] W matrix crosses cores via a host gather between programs):
  P1 (per core): W'-shard [512,3] from host-transposed h-shard (fp16 — fp8
      h.T fails the error budget through the exponential).
  host: concat W'-shards; build wt [3,N] / w4 (W'|ones) layouts; cast
      A-shard.T to fp8 (binary, exact); split h into fp8 hi/lo halves
      (h = h_hi + h_lo, each e4m3; lossless-ish re-encoding).
  P2 (per core): denominators stream with the A.T pieces (at8-stationary
      matmuls, one PSUM accumulation "super-group"), R' chain, then 16
      jc-pair sweeps:
        ct pair (fp16 matmuls, [128,1024] PSUM)
        mt16 = at8 * ct          (DVE, the only full-size PSUM touch)
        mt_hi8 = fp8(mt16)       (ACT copies, a couple on POOL for balance)
        mt_lo8 = mt16 - mt_hi8   (DVE/POOL split)
        psO[ic] += DoubleRow fp8 matmuls: mt_hi.T@h_hi + mt_hi.T@h_lo
                   + mt_lo.T@h_hi   (3-term split => 0.3% rel err, 4x
                   cheaper than fp16 per the 0.5 cycles/row DR rate)
      Warm-up matmuls during the initial load defeat the PE clock ramp.
"""

import numpy as np
import ml_dtypes

import concourse.bass as bass
import concourse.mybir as mybir
import concourse.tile as tile
from concourse import bacc
from concourse import bass_utils

N = 4096
D = 512
H = 3
NCORES = 8
SH = N // NCORES          # 512 output rows per core
JC = N // 128             # 32 j-chunks
IC = SH // 128            # 4 i-chunks
DC = D // 128             # 4 d-chunks
NP = JC // 2              # 16 jc-pairs
F8 = mybir.dt.float8e4
F16 = mybir.dt.float16
F32 = mybir.dt.float32
LN2x4 = float(4.0 * np.log(2.0))   # W scaled by 2^-4 to stay in fp16 range
NP_F8 = ml_dtypes.float8_e4m3
DR = mybir.MatmulPerfMode.DoubleRow


def _body1(tc, hst_in, p_in, w_out):
    """P1: W'-shard [SH,3] from hst [128, IC*DC*128] (h-shard.T, jc-major:
    hst[:, jc, dc, :] = h.T d-chunk dc for j-chunk jc), loaded in 2 pieces.
    The E matmuls use hst as the stationary operand (3-column streams)."""
    nc = tc.nc
    with (
        tc.tile_pool(name="sb1", bufs=1) as sb,
        tc.tile_pool(name="ps1", bufs=1, space="PSUM") as ps,
    ):
        hst = sb.tile([128, IC * DC * 128], F16, tag="hst")
        p16 = sb.tile([128, DC * H], F16, tag="p16")
        wsE = sb.tile([128, IC * H], F16, tag="wsE")
        ebias = sb.tile([128, 1], F32, tag="ebias")
        nc.vector.memset(ebias[:], -LN2x4)
        hst_v = hst[:].rearrange("p (g x) -> g p x", g=2)
        hin_v = hst_in.rearrange("p (g x) -> g p x", g=2)
        for g in range(2):
            nc.sync.dma_start(out=hst_v[g], in_=hin_v[g])
        nc.gpsimd.dma_start(out=p16[:], in_=p_in)

        # one PSUM tile spanning 4 banks: E group per jc, single exp at the end
        psE = ps.tile([128, IC * 512], F32, tag="psE", name="psE")
        for jc in range(IC):
            for dc in range(DC):
                nc.tensor.matmul(
                    psE[:, jc * 512: jc * 512 + H],
                    hst[:, (jc * DC + dc) * 128: (jc * DC + dc + 1) * 128],
                    p16[:, dc * H:(dc + 1) * H],
                    start=(dc == 0),
                    stop=(dc == DC - 1),
                )
        nc.scalar.activation(
            wsE[:].rearrange("p (jc k) -> p jc k", k=H),
            psE[:].rearrange("p (jc x) -> p jc x", x=512)[:, :, 0:H],
            mybir.ActivationFunctionType.Exp,
            bias=ebias[:], scale=1.0,
        )
        nc.vector.tensor_scalar_max(wsE[:], wsE[:], 0.0625)
        nc.sync.dma_start(out=w_out, in_=wsE[:])


def _body2(tc, a8_in, hh_in, hl_in, wt_in, w4_in, id_in, out):
    """P2: denominators + R' chain + 16 jc-pair sweeps with 3-term fp8
    DoubleRow aggregation. a8_in is A-shard.T fp8 [p, jc, i] packed;
    hh_in/hl_in are the fp8 hi/lo splits of h in [p, jc, d] packed order."""
    nc = tc.nc
    mult = mybir.AluOpType.mult
    subop = mybir.AluOpType.subtract

    with (
        tc.tile_pool(name="big", bufs=1) as big,
        tc.tile_pool(name="small", bufs=1) as small,
        tc.tile_pool(name="mtp", bufs=3) as mtp,
        tc.tile_pool(name="osb", bufs=4) as osb,
    ):
        at8 = big.tile([128, JC * SH], F8, tag="at8")       # A.T [p, jc, i]
        hh8 = big.tile([128, JC * D], F8, tag="hh8")        # h hi [p, jc, d]
        hl8 = big.tile([128, JC * D], F8, tag="hl8")        # h lo [p, jc, d]
        wt = small.tile([3, N], F16, tag="wt")              # W'.T
        w4 = small.tile([128, JC * 4], F16, tag="w4")       # W'|ones (j part)
        id16 = small.tile([128, 128], F16, tag="id16")
        scr = small.tile([128, 512], F16, tag="scr")        # warm-up source
        rN = small.tile([128, IC * H], F32, tag="rN")
        rN16 = small.tile([128, IC * H], F16, tag="rN16")   # rowsum/denom'
        rT16 = small.tile([3, SH], F16, tag="rT16")         # R'.T [k, i]

        # ---------------- loads ----------------
        # sync/HWDGE queue order: A.T pieces first (denominators stream with
        # them), then h hi/lo pair-major pieces. Small fp16 tiles on SWDGE.
        NPIECE = 4
        PA = JC // NPIECE                                   # 8 jc per piece
        for pc in range(NPIECE):
            s = pc * PA * SH
            nc.sync.dma_start(out=at8[:, s:s + PA * SH],
                              in_=a8_in[:, s:s + PA * SH])
        nc.gpsimd.dma_start(out=id16[:], in_=id_in)
        nc.gpsimd.dma_start(out=w4[:], in_=w4_in)
        nc.gpsimd.dma_start(out=wt[:], in_=wt_in)
        for pc in range(NPIECE):
            s = pc * PA * D
            nc.sync.dma_start(out=hh8[:, s:s + PA * D],
                              in_=hh_in[:, s:s + PA * D])
            nc.sync.dma_start(out=hl8[:, s:s + PA * D],
                              in_=hl_in[:, s:s + PA * D])

        nc.vector.memset(scr[:], 0.0)
        # warm the ACT table (LoadActFuncSet) off the critical path
        actw = small.tile([1, 2], F16, tag="actw")
        nc.scalar.copy(actw[:], scr[0:1, 0:2])

        with tc.tile_pool(name="pse", bufs=1, space="PSUM") as pse:
            # early PSUM pool: warm-up targets + denominators + R transposes;
            # closed before the sweep pools open so the banks are reused.
            psD2 = pse.tile([128, IC * 4], F32, tag="psd", name="psD2")
            psRT = pse.tile([3, SH], F16, tag="psrt", name="psRT")
            n_warm = 0

            def warm(n):
                nonlocal n_warm
                for _ in range(n):
                    pw = pse.tile([128, 512], F32, tag="warm",
                                  name=f"warm{n_warm}")
                    nc.tensor.matmul(
                        pw[:], scr[:, 0:128], scr[:], start=True, stop=True
                    )
                    n_warm += 1

            warm(10)
            # psD2[p_i, ic*4+k] = sum_j A[i,j] W'[j,k]; k=3 gives rowsum.
            # One accumulation super-group: start only on the very first
            # matmul (pending-zero covers the whole bank region).
            first = True
            for jc in range(JC):
                for ic in range(IC):
                    nc.tensor.matmul(
                        psD2[:, ic * 4:(ic + 1) * 4],
                        at8[:, jc * SH + ic * 128: jc * SH + ic * 128 + 128],
                        w4[:, jc * 4:(jc + 1) * 4],
                        start=first,
                        stop=(jc == JC - 1 and ic == IC - 1),
                        skip_group_check=True,
                    )
                    first = False

            # R' = rowsum * 1/denom', transposed to [k, i]
            psD2_v = psD2[:].rearrange("p (ic s) -> p ic s", s=4)
            nc.vector.reciprocal(
                rN[:].rearrange("p (ic k) -> p ic k", k=H),
                psD2_v[:, :, 0:H],
            )
            for ic in range(IC):
                nc.vector.tensor_scalar(
                    rN16[:, ic * H:(ic + 1) * H], rN[:, ic * H:(ic + 1) * H],
                    psD2[:, ic * 4 + 3: ic * 4 + 4], None, op0=mult,
                )
                nc.tensor.transpose(
                    psRT[:, ic * 128:(ic + 1) * 128],
                    rN16[:, ic * H:(ic + 1) * H],
                    id16[:],
                )
            nc.vector.tensor_copy(rT16[:], psRT[:])

        with (
            tc.tile_pool(name="psc", bufs=2, space="PSUM") as psc,
            tc.tile_pool(name="pso", bufs=1, space="PSUM") as pso,
        ):
            psO = [
                pso.tile([128, D], F32, tag=f"psO{ic}", name=f"psO{ic}")
                for ic in range(IC)
            ]
            hh8_v = hh8[:].rearrange("p (pr two d) -> p pr two d", two=2, d=D)
            hl8_v = hl8[:].rearrange("p (pr two d) -> p pr two d", two=2, d=D)

            # engine assignment per pair for the hi-copy and lo-sub
            cp_eng = ["act"] * NP
            cp_eng[5] = cp_eng[10] = "pool"
            sub_eng = ["dve", "pool"] * (NP // 2)

            for k in range(NP):
                ctp = psc.tile([128, 2 * SH], F32, tag="ctp", name=f"ctp{k}")
                for half in range(2):
                    jc = 2 * k + half
                    nc.tensor.matmul(
                        ctp[:, half * SH:(half + 1) * SH],
                        wt[0:3, jc * 128:(jc + 1) * 128],
                        rT16[:],
                        start=True, stop=True,
                        tile_position=(0, 0),
                    )
                mt16 = mtp.tile([128, 2 * SH], F16, tag="mt16",
                                name=f"mt16_{k}")
                nc.vector.tensor_tensor(
                    mt16[:], at8[:, 2 * k * SH:(2 * k + 2) * SH], ctp[:],
                    op=mult,
                )
                hi8 = mtp.tile([128, 2 * SH], F8, tag="hi8", name=f"hi8_{k}")
                lo8 = mtp.tile([128, 2 * SH], F8, tag="lo8", name=f"lo8_{k}")
                if cp_eng[k] == "act":
                    nc.scalar.copy(hi8[:], mt16[:])
                else:
                    nc.gpsimd.tensor_copy(hi8[:], mt16[:])
                if sub_eng[k] == "dve":
                    nc.vector.tensor_tensor(lo8[:], mt16[:], hi8[:], op=subop)
                else:
                    nc.gpsimd.tensor_tensor(lo8[:], mt16[:], hi8[:], op=subop)

                hi8_v = hi8[:].rearrange("p (two i) -> p two i", two=2)
                lo8_v = lo8[:].rearrange("p (two i) -> p two i", two=2)
                for term, (lt, rt) in enumerate(
                    ((hi8_v, hh8_v), (hi8_v, hl8_v), (lo8_v, hh8_v))
                ):
                    for ic in range(IC):
                        nc.tensor.matmul(
                            psO[ic][:],
                            lt[:, :, ic * 128:(ic + 1) * 128],
                            rt[:, k],
                            start=(k == 0 and term == 0),
                            stop=(k == NP - 1 and term == 2),
                            perf_mode=DR,
                            skip_group_check=True,
                        )

            out_r = out.rearrange("(ic p) d -> ic p d", p=128)
            for ic in range(IC):
                ot = osb.tile([128, D], F16, tag="ot", name=f"ot{ic}")
                if ic % 2 == 0:
                    nc.scalar.copy(ot[:], psO[ic][:])
                else:
                    nc.vector.tensor_copy(ot[:], psO[ic][:])
                (nc.sync if ic % 2 == 0 else nc.gpsimd).dma_start(
                    out=out_r[ic], in_=ot[:]
                )


_CACHE = {}


def _build1():
    if "p1" in _CACHE:
        return _CACHE["p1"]
    nc = bacc.Bacc("TRN2", target_bir_lowering=False, debug=False,
                   num_devices=NCORES)
    hst_in = nc.dram_tensor("hst_in", [128, IC * DC * 128], F16,
                            kind="ExternalInput").ap()
    p_in = nc.dram_tensor("p_in", [128, DC * H], F16, kind="ExternalInput").ap()
    w_out = nc.dram_tensor("w_out", [128, IC * H], F16,
                           kind="ExternalOutput").ap()
    with tile.TileContext(nc) as tc:
        _body1(tc, hst_in, p_in, w_out)
    nc.compile()
    _CACHE["p1"] = nc
    return nc


def _build2():
    if "p2" in _CACHE:
        return _CACHE["p2"]
    nc = bacc.Bacc("TRN2", target_bir_lowering=False, debug=False,
                   num_devices=NCORES)
    a8_in = nc.dram_tensor("a8_in", [128, JC * SH], F8,
                           kind="ExternalInput").ap()
    hh_in = nc.dram_tensor("hh_in", [128, JC * D], F8,
                           kind="ExternalInput").ap()
    hl_in = nc.dram_tensor("hl_in", [128, JC * D], F8,
                           kind="ExternalInput").ap()
    wt_in = nc.dram_tensor("wt_in", [3, N], F16, kind="ExternalInput").ap()
    w4_in = nc.dram_tensor("w4_in", [128, JC * 4], F16,
                           kind="ExternalInput").ap()
    id_in = nc.dram_tensor("id_in", [128, 128], F16, kind="ExternalInput").ap()
    out = nc.dram_tensor("out", [SH, D], F16, kind="ExternalOutput").ap()
    with tile.TileContext(nc) as tc:
        _body2(tc, a8_in, hh_in, hl_in, wt_in, w4_in, id_in, out)
    nc.compile()
    _CACHE["p2"] = nc
    return nc


def kernel(graph_info, h, P, _trace=False, _results_out=None):
    graph_info = np.ascontiguousarray(graph_info, dtype=np.float32)
    h = np.ascontiguousarray(h, dtype=np.float32)
    P = np.ascontiguousarray(P, dtype=np.float32)
    nc1 = _build1()
    nc2 = _build2()

    # host-side shard/layout prep (pure data movement + dtype casts)
    h16_full = h.astype(np.float16)
    p16_host = np.ascontiguousarray(
        P.astype(np.float16).reshape(DC, 128, H).transpose(1, 0, 2)
    ).reshape(128, DC * H)
    in1 = []
    for c in range(NCORES):
        hsT = h16_full[c * SH:(c + 1) * SH, :].T  # [D, SH]
        hst_host = np.ascontiguousarray(
            hsT.reshape(DC, 128, IC, 128).transpose(1, 2, 0, 3)
        ).reshape(128, IC * DC * 128)
        in1.append({"hst_in": hst_host, "p_in": p16_host})
    res1 = bass_utils.run_bass_kernel_spmd(
        nc1, in1, core_ids=list(range(NCORES)), trace=_trace
    )
    w_full = np.concatenate(
        [
            res1.results[c]["w_out"]
            .reshape(128, IC, H).transpose(1, 0, 2).reshape(SH, H)
            for c in range(NCORES)
        ],
        axis=0,
    )  # [N, 3] fp16, scaled by 2^-4

    wt_host = np.ascontiguousarray(w_full.T)  # [3, N]
    w4_host = np.ascontiguousarray(
        np.concatenate(
            [w_full.reshape(JC, 128, H).transpose(1, 0, 2),
             np.ones((128, JC, 1), np.float16)],
            axis=2,
        ).reshape(128, JC * 4)
    )
    id_host = np.eye(128, dtype=np.float16)

    # fp8 hi/lo split of h (host-side re-encoding; h = hh + hl up to e4m3^2)
    h_hi = np.clip(h, -240, 240).astype(NP_F8)
    h_lo = (h - h_hi.astype(np.float32)).astype(NP_F8)
    hh_host = np.ascontiguousarray(
        h_hi.reshape(JC, 128, D).transpose(1, 0, 2)).reshape(128, JC * D)
    hl_host = np.ascontiguousarray(
        h_lo.reshape(JC, 128, D).transpose(1, 0, 2)).reshape(128, JC * D)

    in2 = []
    for c in range(NCORES):
        at = np.ascontiguousarray(
            graph_info[c * SH:(c + 1) * SH, :].T
        ).astype(NP_F8)                      # [N(j), SH(i)]
        a8_host = np.ascontiguousarray(
            at.reshape(JC, 128, SH).transpose(1, 0, 2)).reshape(128, JC * SH)
        in2.append({
            "a8_in": a8_host,
            "hh_in": hh_host,
            "hl_in": hl_host,
            "wt_in": wt_host,
            "w4_in": w4_host,
            "id_in": id_host,
        })
    res2 = bass_utils.run_bass_kernel_spmd(
        nc2, in2, core_ids=list(range(NCORES)), trace=_trace
    )
    if _results_out is not None:
        _results_out.extend([res1, res2])
    return np.concatenate(
        [res2.results[c]["out"].astype(np.float32) for c in range(NCORES)],
        axis=0,
    )


# revision 8
# speedup vs baseline: 1.0809x; 1.0809x over previous
"""GAT-style attention (gnn_message_passing) Trainium2 kernel, 8-core row-parallel.

Math (algebraically identical to the reference masked-softmax attention):
  E = relu(h @ P)                 [N,3]
  W' = max(exp(E - 4ln2), 1/16)   (= exp(relu(E))/16, fp16-safe range)
  denom'[i,k] = sum_j A[i,j] W'[j,k]   (k=3 slot sums ones -> rowsum[i])
  R'[i,k] = rowsum[i] / denom'[i,k]
  ct[j,i]  = sum_k W'[j,k] R'[i,k] = rowsum[i] * C[i,j]
  mt[j,i]  = A[i,j] * ct[j,i]
  out[i,:] = sum_j mt[j,i] h[j,:]

Two SPMD programs (cost-modeled collectives are ~15us fixed -> too slow; the
tiny [4096,3] W matrix crosses cores via a host gather between programs):
  P1 (per core): W'-shard [512,3] from host-transposed h-shard (fp16 — fp8
      h.T fails the error budget through the exponential).
  host: concat W'-shards; build wt [3,N] / w4 (W'|ones) layouts; cast
      A-shard.T to fp8 (binary, exact); split h into fp8 hi/lo halves
      (h = h_hi + h_lo, each e4m3; lossless-ish re-encoding).
  P2 (per core): denominators stream with the A.T pieces (at8-stationary
      matmuls, one PSUM accumulation "super-group"), R' chain, then 16
      jc-pair sweeps:
        ct pair (fp16 matmuls, [128,1024] PSUM)
        mt16 = at8 * ct          (DVE, the only full-size PSUM touch)
        mt_hi8 = fp8(mt16)       (ACT copies, a couple on POOL for balance)
        mt_lo8 = mt16 - mt_hi8   (DVE/POOL split)
        psO[ic] += DoubleRow fp8 matmuls: mt_hi.T@h_hi + mt_hi.T@h_lo
                   + mt_lo.T@h_hi   (3-term split => 0.3% rel err, 4x
                   cheaper than fp16 per the 0.5 cycles/row DR rate)
      Warm-up matmuls during the initial load defeat the PE clock ramp.
"""

import numpy as np
import ml_dtypes

import concourse.bass as bass
import concourse.mybir as mybir
import concourse.tile as tile
from concourse import bacc
from concourse import bass_utils

N = 4096
D = 512
H = 3
NCORES = 8
SH = N // NCORES          # 512 output rows per core
JC = N // 128             # 32 j-chunks
IC = SH // 128            # 4 i-chunks
DC = D // 128             # 4 d-chunks
NP = JC // 2              # 16 jc-pairs
F8 = mybir.dt.float8e4
F16 = mybir.dt.float16
F32 = mybir.dt.float32
LN2x4 = float(4.0 * np.log(2.0))   # W scaled by 2^-4 to stay in fp16 range
NP_F8 = ml_dtypes.float8_e4m3
DR = mybir.MatmulPerfMode.DoubleRow


def _body1(tc, hst_in, p_in, w_out):
    """P1: W'-shard [SH,3] from hst [128, IC*DC*128] (h-shard.T, jc-major:
    hst[:, jc, dc, :] = h.T d-chunk dc for j-chunk jc), loaded in 2 pieces.
    The E matmuls use hst as the stationary operand (3-column streams)."""
    nc = tc.nc
    with (
        tc.tile_pool(name="sb1", bufs=1) as sb,
        tc.tile_pool(name="ps1", bufs=1, space="PSUM") as ps,
    ):
        hst = sb.tile([128, IC * DC * 128], F16, tag="hst")
        p16 = sb.tile([128, DC * H], F16, tag="p16")
        wsE = sb.tile([128, IC * H], F16, tag="wsE")
        ebias = sb.tile([128, 1], F32, tag="ebias")
        nc.vector.memset(ebias[:], -LN2x4)
        hst_v = hst[:].rearrange("p (g x) -> g p x", g=2)
        hin_v = hst_in.rearrange("p (g x) -> g p x", g=2)
        for g in range(2):
            nc.sync.dma_start(out=hst_v[g], in_=hin_v[g])
        nc.gpsimd.dma_start(out=p16[:], in_=p_in)

        # one PSUM tile spanning 4 banks: E group per jc, single exp at the end
        psE = ps.tile([128, IC * 512], F32, tag="psE", name="psE")
        for jc in range(IC):
            for dc in range(DC):
                nc.tensor.matmul(
                    psE[:, jc * 512: jc * 512 + H],
                    hst[:, (jc * DC + dc) * 128: (jc * DC + dc + 1) * 128],
                    p16[:, dc * H:(dc + 1) * H],
                    start=(dc == 0),
                    stop=(dc == DC - 1),
                )
        nc.scalar.activation(
            wsE[:].rearrange("p (jc k) -> p jc k", k=H),
            psE[:].rearrange("p (jc x) -> p jc x", x=512)[:, :, 0:H],
            mybir.ActivationFunctionType.Exp,
            bias=ebias[:], scale=1.0,
        )
        nc.vector.tensor_scalar_max(wsE[:], wsE[:], 0.0625)
        nc.sync.dma_start(out=w_out, in_=wsE[:])


def _body2(tc, a8_in, hh_in, hl_in, wt_in, w4_in, id_in, out):
    """P2: denominators + R' chain + 16 jc-pair sweeps with 3-term fp8
    DoubleRow aggregation. a8_in is A-shard.T fp8 [p, jc, i] packed;
    hh_in/hl_in are the fp8 hi/lo splits of h in [p, jc, d] packed order."""
    nc = tc.nc
    mult = mybir.AluOpType.mult
    subop = mybir.AluOpType.subtract

    with (
        tc.tile_pool(name="big", bufs=1) as big,
        tc.tile_pool(name="small", bufs=1) as small,
        tc.tile_pool(name="mtp", bufs=4) as mtp,
        tc.tile_pool(name="osb", bufs=4) as osb,
    ):
        at8 = big.tile([128, JC * SH], F8, tag="at8")       # A.T [p, jc, i]
        hh8 = big.tile([128, JC * D], F8, tag="hh8")        # h hi [p, jc, d]
        hl8 = big.tile([128, JC * D], F8, tag="hl8")        # h lo [p, jc, d]
        wt = small.tile([3, N], F16, tag="wt")              # W'.T
        w4 = small.tile([128, JC * 4], F16, tag="w4")       # W'|ones (j part)
        id16 = small.tile([128, 128], F16, tag="id16")
        scr = small.tile([128, 512], F16, tag="scr")        # warm-up source
        rN = small.tile([128, IC * H], F32, tag="rN")
        rN16 = small.tile([128, IC * H], F16, tag="rN16")   # rowsum/denom'
        rT16 = small.tile([3, SH], F16, tag="rT16")         # R'.T [k, i]

        # ---------------- loads ----------------
        # sync/HWDGE queue order: tiny tensors, then A.T pieces (denominators
        # stream with them), then h hi/lo pair-major pieces.
        nc.sync.dma_start(out=id16[:], in_=id_in)
        nc.sync.dma_start(out=w4[:], in_=w4_in)
        nc.sync.dma_start(out=wt[:], in_=wt_in)
        NPIECE = 8
        PA = JC // NPIECE                                   # 4 jc per piece
        for pc in range(NPIECE):
            s = pc * PA * SH
            nc.sync.dma_start(out=at8[:, s:s + PA * SH],
                              in_=a8_in[:, s:s + PA * SH])
        for pc in range(4):
            s = pc * (JC // 4) * D
            nc.sync.dma_start(out=hh8[:, s:s + (JC // 4) * D],
                              in_=hh_in[:, s:s + (JC // 4) * D])
            nc.sync.dma_start(out=hl8[:, s:s + (JC // 4) * D],
                              in_=hl_in[:, s:s + (JC // 4) * D])

        nc.vector.memset(scr[:], 0.0)
        # warm the ACT table (LoadActFuncSet) off the critical path
        actw = small.tile([1, 2], F16, tag="actw")
        nc.scalar.copy(actw[:], scr[0:1, 0:2])

        with tc.tile_pool(name="pse", bufs=1, space="PSUM") as pse:
            # early PSUM pool: warm-up targets + denominators + R transposes;
            # closed before the sweep pools open so the banks are reused.
            psD2 = pse.tile([128, IC * 4], F32, tag="psd", name="psD2")
            psRT = pse.tile([3, SH], F16, tag="psrt", name="psRT")
            n_warm = 0

            def warm(n):
                nonlocal n_warm
                for _ in range(n):
                    pw = pse.tile([128, 512], F32, tag="warm",
                                  name=f"warm{n_warm}")
                    nc.tensor.matmul(
                        pw[:], scr[:, 0:128], scr[:], start=True, stop=True
                    )
                    n_warm += 1

            # psD2[p_i, ic*4+k] = sum_j A[i,j] W'[j,k]; k=3 gives rowsum.
            # One accumulation super-group: start only on the very first
            # matmul (pending-zero covers the whole bank region). Denominator
            # matmuls stream with the at8 pieces; warm-up matmuls fill the
            # PE gaps between pieces so the clock ramp isn't reset.
            warm(2)
            first = True
            for pc in range(NPIECE):
                for jc in range(pc * PA, (pc + 1) * PA):
                    for ic in range(IC):
                        nc.tensor.matmul(
                            psD2[:, ic * 4:(ic + 1) * 4],
                            at8[:, jc * SH + ic * 128:
                                jc * SH + ic * 128 + 128],
                            w4[:, jc * 4:(jc + 1) * 4],
                            start=first,
                            stop=(jc == JC - 1 and ic == IC - 1),
                            skip_group_check=True,
                        )
                        first = False
                if pc < NPIECE - 1:
                    warm(2)

            # R' = rowsum * 1/denom', transposed to [k, i]
            psD2_v = psD2[:].rearrange("p (ic s) -> p ic s", s=4)
            nc.vector.reciprocal(
                rN[:].rearrange("p (ic k) -> p ic k", k=H),
                psD2_v[:, :, 0:H],
            )
            for ic in range(IC):
                nc.vector.tensor_scalar(
                    rN16[:, ic * H:(ic + 1) * H], rN[:, ic * H:(ic + 1) * H],
                    psD2[:, ic * 4 + 3: ic * 4 + 4], None, op0=mult,
                )
                nc.tensor.transpose(
                    psRT[:, ic * 128:(ic + 1) * 128],
                    rN16[:, ic * H:(ic + 1) * H],
                    id16[:],
                )
            nc.vector.tensor_copy(rT16[:], psRT[:])

        with (
            tc.tile_pool(name="psc", bufs=2, space="PSUM") as psc,
            tc.tile_pool(name="pso", bufs=1, space="PSUM") as pso,
        ):
            psO = [
                pso.tile([128, D], F32, tag=f"psO{ic}", name=f"psO{ic}")
                for ic in range(IC)
            ]
            hh8_v = hh8[:].rearrange("p (pr two d) -> p pr two d", two=2, d=D)
            hl8_v = hl8[:].rearrange("p (pr two d) -> p pr two d", two=2, d=D)

            # engine assignment per pair for the hi-copy and lo-sub
            cp_eng = ["act"] * NP
            sub_eng = (["dve", "pool", "pool"] * NP)[:NP]

            his = {}
            los = {}

            def front(k):
                # ct pair -> mask -> hi copy -> lo sub for pair k
                ctp = psc.tile([128, 2 * SH], F32, tag="ctp", name=f"ctp{k}")
                for half in range(2):
                    jc = 2 * k + half
                    nc.tensor.matmul(
                        ctp[:, half * SH:(half + 1) * SH],
                        wt[0:3, jc * 128:(jc + 1) * 128],
                        rT16[:],
                        start=True, stop=True,
                        tile_position=(0, 0),
                    )
                mt16 = mtp.tile([128, 2 * SH], F16, tag="mt16",
                                name=f"mt16_{k}")
                nc.vector.tensor_tensor(
                    mt16[:], at8[:, 2 * k * SH:(2 * k + 2) * SH], ctp[:],
                    op=mult,
                )
                hi8 = mtp.tile([128, 2 * SH], F8, tag="hi8", name=f"hi8_{k}")
                lo8 = mtp.tile([128, 2 * SH], F8, tag="lo8", name=f"lo8_{k}")
                if cp_eng[k] == "act":
                    nc.scalar.copy(hi8[:], mt16[:])
                else:
                    nc.gpsimd.tensor_copy(hi8[:], mt16[:])
                if sub_eng[k] == "dve":
                    nc.vector.tensor_tensor(lo8[:], mt16[:], hi8[:], op=subop)
                else:
                    nc.gpsimd.tensor_tensor(lo8[:], mt16[:], hi8[:], op=subop)
                his[k], los[k] = hi8, lo8

            def agg(k):
                hi8_v = his[k][:].rearrange("p (two i) -> p two i", two=2)
                lo8_v = los[k][:].rearrange("p (two i) -> p two i", two=2)
                for term, (lt, rt) in enumerate(
                    ((hi8_v, hh8_v), (hi8_v, hl8_v), (lo8_v, hh8_v))
                ):
                    for ic in range(IC):
                        nc.tensor.matmul(
                            psO[ic][:],
                            lt[:, :, ic * 128:(ic + 1) * 128],
                            rt[:, k],
                            start=(k == 0 and term == 0),
                            stop=(k == NP - 1 and term == 2),
                            perf_mode=DR,
                            skip_group_check=True,
                        )

            # software pipeline: the front chain (PE ct -> DVE mask -> ACT
            # copy -> DVE/POOL sub) runs LAG pairs ahead of the aggregation
            # so the per-pair cross-engine latency never stalls the PE.
            LAG = 2
            for k in range(NP + LAG):
                if k < NP:
                    front(k)
                if k >= LAG:
                    agg(k - LAG)

            out_r = out.rearrange("(ic p) d -> ic p d", p=128)
            for ic in range(IC):
                ot = osb.tile([128, D], F16, tag="ot", name=f"ot{ic}")
                if ic % 2 == 0:
                    nc.scalar.copy(ot[:], psO[ic][:])
                else:
                    nc.vector.tensor_copy(ot[:], psO[ic][:])
                (nc.sync if ic % 2 == 0 else nc.scalar).dma_start(
                    out=out_r[ic], in_=ot[:]
                )


_CACHE = {}


def _build1():
    if "p1" in _CACHE:
        return _CACHE["p1"]
    nc = bacc.Bacc("TRN2", target_bir_lowering=False, debug=False,
                   num_devices=NCORES)
    hst_in = nc.dram_tensor("hst_in", [128, IC * DC * 128], F16,
                            kind="ExternalInput").ap()
    p_in = nc.dram_tensor("p_in", [128, DC * H], F16, kind="ExternalInput").ap()
    w_out = nc.dram_tensor("w_out", [128, IC * H], F16,
                           kind="ExternalOutput").ap()
    with tile.TileContext(nc) as tc:
        _body1(tc, hst_in, p_in, w_out)
    nc.compile()
    _CACHE["p1"] = nc
    return nc


def _build2():
    if "p2" in _CACHE:
        return _CACHE["p2"]
    nc = bacc.Bacc("TRN2", target_bir_lowering=False, debug=False,
                   num_devices=NCORES)
    a8_in = nc.dram_tensor("a8_in", [128, JC * SH], F8,
                           kind="ExternalInput").ap()
    hh_in = nc.dram_tensor("hh_in", [128, JC * D], F8,
                           kind="ExternalInput").ap()
    hl_in = nc.dram_tensor("hl_in", [128, JC * D], F8,
                           kind="ExternalInput").ap()
    wt_in = nc.dram_tensor("wt_in", [3, N], F16, kind="ExternalInput").ap()
    w4_in = nc.dram_tensor("w4_in", [128, JC * 4], F16,
                           kind="ExternalInput").ap()
    id_in = nc.dram_tensor("id_in", [128, 128], F16, kind="ExternalInput").ap()
    out = nc.dram_tensor("out", [SH, D], F16, kind="ExternalOutput").ap()
    with tile.TileContext(nc) as tc:
        _body2(tc, a8_in, hh_in, hl_in, wt_in, w4_in, id_in, out)
    nc.compile()
    _CACHE["p2"] = nc
    return nc


def kernel(graph_info, h, P, _trace=False, _results_out=None):
    graph_info = np.ascontiguousarray(graph_info, dtype=np.float32)
    h = np.ascontiguousarray(h, dtype=np.float32)
    P = np.ascontiguousarray(P, dtype=np.float32)
    nc1 = _build1()
    nc2 = _build2()

    # host-side shard/layout prep (pure data movement + dtype casts)
    h16_full = h.astype(np.float16)
    p16_host = np.ascontiguousarray(
        P.astype(np.float16).reshape(DC, 128, H).transpose(1, 0, 2)
    ).reshape(128, DC * H)
    in1 = []
    for c in range(NCORES):
        hsT = h16_full[c * SH:(c + 1) * SH, :].T  # [D, SH]
        hst_host = np.ascontiguousarray(
            hsT.reshape(DC, 128, IC, 128).transpose(1, 2, 0, 3)
        ).reshape(128, IC * DC * 128)
        in1.append({"hst_in": hst_host, "p_in": p16_host})
    res1 = bass_utils.run_bass_kernel_spmd(
        nc1, in1, core_ids=list(range(NCORES)), trace=_trace
    )
    w_full = np.concatenate(
        [
            res1.results[c]["w_out"]
            .reshape(128, IC, H).transpose(1, 0, 2).reshape(SH, H)
            for c in range(NCORES)
        ],
        axis=0,
    )  # [N, 3] fp16, scaled by 2^-4

    wt_host = np.ascontiguousarray(w_full.T)  # [3, N]
    w4_host = np.ascontiguousarray(
        np.concatenate(
            [w_full.reshape(JC, 128, H).transpose(1, 0, 2),
             np.ones((128, JC, 1), np.float16)],
            axis=2,
        ).reshape(128, JC * 4)
    )
    id_host = np.eye(128, dtype=np.float16)

    # fp8 hi/lo split of h (host-side re-encoding; h = hh + hl up to e4m3^2)
    h_hi = np.clip(h, -240, 240).astype(NP_F8)
    h_lo = (h - h_hi.astype(np.float32)).astype(NP_F8)
    hh_host = np.ascontiguousarray(
        h_hi.reshape(JC, 128, D).transpose(1, 0, 2)).reshape(128, JC * D)
    hl_host = np.ascontiguousarray(
        h_lo.reshape(JC, 128, D).transpose(1, 0, 2)).reshape(128, JC * D)

    in2 = []
    for c in range(NCORES):
        at = np.ascontiguousarray(
            graph_info[c * SH:(c + 1) * SH, :].T
        ).astype(NP_F8)                      # [N(j), SH(i)]
        a8_host = np.ascontiguousarray(
            at.reshape(JC, 128, SH).transpose(1, 0, 2)).reshape(128, JC * SH)
        in2.append({
            "a8_in": a8_host,
            "hh_in": hh_host,
            "hl_in": hl_host,
            "wt_in": wt_host,
            "w4_in": w4_host,
            "id_in": id_host,
        })
    res2 = bass_utils.run_bass_kernel_spmd(
        nc2, in2, core_ids=list(range(NCORES)), trace=_trace
    )
    if _results_out is not None:
        _results_out.extend([res1, res2])
    return np.concatenate(
        [res2.results[c]["out"].astype(np.float32) for c in range(NCORES)],
        axis=0,
    )


# revision 13
# speedup vs baseline: 1.1300x; 1.0454x over previous
"""GAT-style attention (gnn_message_passing) Trainium2 kernel, 8-core row-parallel.

Math (algebraically identical to the reference masked-softmax attention):
  E = relu(h @ P)                 [N,3]
  W' = max(exp(E - 4ln2), 1/16)   (= exp(relu(E))/16, fp16-safe range)
  denom'[i,k] = sum_j A[i,j] W'[j,k]   (k=3 slot sums ones -> rowsum[i])
  R'[i,k] = rowsum[i] / denom'[i,k]
  ct[j,i]  = sum_k W'[j,k] R'[i,k] = rowsum[i] * C[i,j]
  mt[j,i]  = A[i,j] * ct[j,i]
  out[i,:] = sum_j mt[j,i] h[j,:]

Two SPMD programs (cost-modeled collectives are ~15us fixed -> too slow; the
tiny [4096,3] W matrix crosses cores via a host gather between programs):
  P1 (per core): W'-shard [512,3] from host-transposed h-shard (fp16 — fp8
      h.T fails the error budget through the exponential).
  host: concat W'-shards; build wt [3,N] / w4 (W'|ones) layouts; cast
      A-shard.T to fp8 (binary, exact); split h into fp8 hi/lo halves
      (h = h_hi + h_lo, each e4m3; lossless-ish re-encoding).
  P2 (per core): denominators stream with the A.T pieces (at8-stationary
      matmuls, one PSUM accumulation "super-group"), R' chain, then 16
      jc-pair sweeps:
        ct pair (fp16 matmuls, [128,1024] PSUM)
        mt16 = at8 * ct          (DVE, the only full-size PSUM touch)
        mt_hi8 = fp8(mt16)       (ACT copies, a couple on POOL for balance)
        mt_lo8 = mt16 - mt_hi8   (DVE/POOL split)
        psO[ic] += DoubleRow fp8 matmuls: mt_hi.T@h_hi + mt_hi.T@h_lo
                   + mt_lo.T@h_hi   (3-term split => 0.3% rel err, 4x
                   cheaper than fp16 per the 0.5 cycles/row DR rate)
      Warm-up matmuls during the initial load defeat the PE clock ramp.
"""

import numpy as np
import ml_dtypes

import concourse.bass as bass
import concourse.mybir as mybir
import concourse.tile as tile
from concourse import bacc
from concourse import bass_utils

N = 4096
D = 512
H = 3
NCORES = 8
SH = N // NCORES          # 512 output rows per core
JC = N // 128             # 32 j-chunks
IC = SH // 128            # 4 i-chunks
DC = D // 128             # 4 d-chunks
NP = JC // 2              # 16 jc-pairs
F8 = mybir.dt.float8e4
F16 = mybir.dt.float16
F32 = mybir.dt.float32
LN2x4 = float(4.0 * np.log(2.0))   # W scaled by 2^-4 to stay in fp16 range
NP_F8 = ml_dtypes.float8_e4m3
DR = mybir.MatmulPerfMode.DoubleRow


def _body1(tc, hst_in, p_in, w_out):
    """P1: W'-shard [SH,3] from hst [128, IC*DC*128] (h-shard.T, jc-major:
    hst[:, jc, dc, :] = h.T d-chunk dc for j-chunk jc), loaded in 2 pieces.
    The E matmuls use hst as the stationary operand (3-column streams)."""
    nc = tc.nc
    with (
        tc.tile_pool(name="sb1", bufs=1) as sb,
        tc.tile_pool(name="ps1", bufs=1, space="PSUM") as ps,
    ):
        hst = sb.tile([128, IC * DC * 128], F16, tag="hst")
        p16 = sb.tile([128, DC * H], F16, tag="p16")
        wsE = sb.tile([128, IC * H], F16, tag="wsE")
        ebias = sb.tile([128, 1], F32, tag="ebias")
        nc.vector.memset(ebias[:], -LN2x4)
        hst_v = hst[:].rearrange("p (g x) -> g p x", g=2)
        hin_v = hst_in.rearrange("p (g x) -> g p x", g=2)
        for g in range(2):
            nc.sync.dma_start(out=hst_v[g], in_=hin_v[g])
        nc.gpsimd.dma_start(out=p16[:], in_=p_in)

        # one PSUM tile spanning 4 banks: E group per jc, single exp at the end
        psE = ps.tile([128, IC * 512], F32, tag="psE", name="psE")
        for jc in range(IC):
            for dc in range(DC):
                nc.tensor.matmul(
                    psE[:, jc * 512: jc * 512 + H],
                    hst[:, (jc * DC + dc) * 128: (jc * DC + dc + 1) * 128],
                    p16[:, dc * H:(dc + 1) * H],
                    start=(dc == 0),
                    stop=(dc == DC - 1),
                )
        nc.scalar.activation(
            wsE[:].rearrange("p (jc k) -> p jc k", k=H),
            psE[:].rearrange("p (jc x) -> p jc x", x=512)[:, :, 0:H],
            mybir.ActivationFunctionType.Exp,
            bias=ebias[:], scale=1.0,
        )
        nc.vector.tensor_scalar_max(wsE[:], wsE[:], 0.0625)
        nc.sync.dma_start(out=w_out, in_=wsE[:])


def _body2(tc, a8_in, hh_in, hl_in, wt_in, w4_in, id_in, out):
    """P2: denominators + R' chain + 16 jc-pair sweeps with 3-term fp8
    DoubleRow aggregation. a8_in is A-shard.T fp8 [p, jc, i] packed;
    hh_in/hl_in are the fp8 hi/lo splits of h in [p, jc, d] packed order."""
    nc = tc.nc
    mult = mybir.AluOpType.mult
    subop = mybir.AluOpType.subtract

    with (
        tc.tile_pool(name="big", bufs=1) as big,
        tc.tile_pool(name="small", bufs=1) as small,
        tc.tile_pool(name="mtp", bufs=5) as mtp,
        tc.tile_pool(name="osb", bufs=4) as osb,
    ):
        at8 = big.tile([128, JC * SH], F8, tag="at8")       # A.T [p, jc, i]
        hh8 = big.tile([128, JC * D], F8, tag="hh8")        # h hi [p, jc, d]
        hl8 = big.tile([128, JC * D], F8, tag="hl8")        # h lo [p, jc, d]
        wt = small.tile([3, N], F16, tag="wt")              # W'.T
        w4 = small.tile([128, JC * 4], F16, tag="w4")       # W'|ones (j part)
        id16 = small.tile([128, 128], F16, tag="id16")
        scr = small.tile([128, 512], F16, tag="scr")        # warm-up source
        rN = small.tile([128, IC * H], F32, tag="rN")
        rN16 = small.tile([128, IC * H], F16, tag="rN16")   # rowsum/denom'
        rT16 = small.tile([3, SH], F16, tag="rT16")         # R'.T [k, i]

        # ---------------- loads ----------------
        # sync/HWDGE queue: A.T pieces first (denominators stream with them),
        # then h hi/lo pair-major pieces. Tiny tensors go on the scalar
        # HWDGE queue so they don't delay the at8 stream's SP-SEQ issue.
        nc.scalar.dma_start(out=w4[:], in_=w4_in)
        nc.scalar.dma_start(out=id16[:], in_=id_in)
        nc.scalar.dma_start(out=wt[:], in_=wt_in)
        NPIECE = 4
        PA = JC // NPIECE                                   # 8 jc per piece
        for pc in range(NPIECE):
            s = pc * PA * SH
            nc.sync.dma_start(out=at8[:, s:s + PA * SH],
                              in_=a8_in[:, s:s + PA * SH])
        for pc in range(4):
            s = pc * (JC // 4) * D
            nc.sync.dma_start(out=hh8[:, s:s + (JC // 4) * D],
                              in_=hh_in[:, s:s + (JC // 4) * D])
            nc.sync.dma_start(out=hl8[:, s:s + (JC // 4) * D],
                              in_=hl_in[:, s:s + (JC // 4) * D])

        nc.vector.memset(scr[:], 0.0)
        # warm the ACT table (LoadActFuncSet) off the critical path
        actw = small.tile([1, 2], F16, tag="actw")
        nc.scalar.copy(actw[:], scr[0:1, 0:2])

        with tc.tile_pool(name="pse", bufs=1, space="PSUM") as pse:
            # early PSUM pool: warm-up targets + denominators + R transposes;
            # closed before the sweep pools open so the banks are reused.
            psD2 = pse.tile([128, IC * 4], F32, tag="psd", name="psD2")
            psRT = pse.tile([3, SH], F16, tag="psrt", name="psRT")
            n_warm = 0

            def warm(n):
                nonlocal n_warm
                for _ in range(n):
                    pw = pse.tile([128, 512], F32, tag="warm",
                                  name=f"warm{n_warm}")
                    nc.tensor.matmul(
                        pw[:], scr[:, 0:128], scr[:], start=True, stop=True
                    )
                    n_warm += 1

            # psD2[p_i, ic*4+k] = sum_j A[i,j] W'[j,k]; k=3 gives rowsum.
            # One accumulation super-group: start only on the very first
            # matmul (pending-zero covers the whole bank region). Denominator
            # matmuls stream with the at8 pieces; warm-up matmuls fill the
            # PE gaps between pieces so the clock ramp isn't reset.
            warm(2)
            first = True
            for pc in range(NPIECE):
                for jc in range(pc * PA, (pc + 1) * PA):
                    for ic in range(IC):
                        nc.tensor.matmul(
                            psD2[:, ic * 4:(ic + 1) * 4],
                            at8[:, jc * SH + ic * 128:
                                jc * SH + ic * 128 + 128],
                            w4[:, jc * 4:(jc + 1) * 4],
                            start=first,
                            stop=(jc == JC - 1 and ic == IC - 1),
                            skip_group_check=True,
                        )
                        first = False
                if pc < NPIECE - 1:
                    warm(2)

            # R' = rowsum * 1/denom', transposed to [k, i]
            psD2_v = psD2[:].rearrange("p (ic s) -> p ic s", s=4)
            nc.vector.reciprocal(
                rN[:].rearrange("p (ic k) -> p ic k", k=H),
                psD2_v[:, :, 0:H],
            )
            for ic in range(IC):
                nc.vector.tensor_scalar(
                    rN16[:, ic * H:(ic + 1) * H], rN[:, ic * H:(ic + 1) * H],
                    psD2[:, ic * 4 + 3: ic * 4 + 4], None, op0=mult,
                )
                nc.tensor.transpose(
                    psRT[:, ic * 128:(ic + 1) * 128],
                    rN16[:, ic * H:(ic + 1) * H],
                    id16[:],
                )
            nc.vector.tensor_copy(rT16[:], psRT[:])

        with (
            tc.tile_pool(name="psc", bufs=2, space="PSUM") as psc,
            tc.tile_pool(name="pso", bufs=1, space="PSUM") as pso,
        ):
            psO = [
                pso.tile([128, D], F32, tag=f"psO{ic}", name=f"psO{ic}")
                for ic in range(IC)
            ]
            hh8_v = hh8[:].rearrange("p (pr two d) -> p pr two d", two=2, d=D)
            hl8_v = hl8[:].rearrange("p (pr two d) -> p pr two d", two=2, d=D)

            # engine assignment per pair for the hi-copy and lo-sub
            cp_eng = ["act"] * NP
            sub_eng = (["dve", "pool"] * NP)[:NP]
            sub_eng[14] = "pool"

            his = {}
            los = {}

            def front(k):
                # ct pair -> mask -> hi copy -> lo sub for pair k
                ctp = psc.tile([128, 2 * SH], F32, tag="ctp", name=f"ctp{k}")
                for half in range(2):
                    jc = 2 * k + half
                    nc.tensor.matmul(
                        ctp[:, half * SH:(half + 1) * SH],
                        wt[0:3, jc * 128:(jc + 1) * 128],
                        rT16[:],
                        start=True, stop=True,
                        tile_position=(0, 0),
                    )
                mt16 = mtp.tile([128, 2 * SH], F16, tag="mt16",
                                name=f"mt16_{k}")
                nc.vector.tensor_tensor(
                    mt16[:], at8[:, 2 * k * SH:(2 * k + 2) * SH], ctp[:],
                    op=mult,
                )
                hi8 = mtp.tile([128, 2 * SH], F8, tag="hi8", name=f"hi8_{k}")
                lo8 = mtp.tile([128, 2 * SH], F8, tag="lo8", name=f"lo8_{k}")
                if cp_eng[k] == "act":
                    nc.scalar.copy(hi8[:], mt16[:])
                else:
                    nc.gpsimd.tensor_copy(hi8[:], mt16[:])
                if sub_eng[k] == "dve":
                    nc.vector.tensor_tensor(lo8[:], mt16[:], hi8[:], op=subop)
                else:
                    nc.gpsimd.tensor_tensor(lo8[:], mt16[:], hi8[:], op=subop)
                his[k], los[k] = hi8, lo8

            def agg(k):
                hi8_v = his[k][:].rearrange("p (two i) -> p two i", two=2)
                lo8_v = los[k][:].rearrange("p (two i) -> p two i", two=2)
                for term, (lt, rt) in enumerate(
                    ((hi8_v, hh8_v), (hi8_v, hl8_v), (lo8_v, hh8_v))
                ):
                    for ic in range(IC):
                        nc.tensor.matmul(
                            psO[ic][:],
                            lt[:, :, ic * 128:(ic + 1) * 128],
                            rt[:, k],
                            start=(k == 0 and term == 0),
                            stop=(k == NP - 1 and term == 2),
                            perf_mode=DR,
                            skip_group_check=True,
                        )

            # software pipeline: the front chain (PE ct -> DVE mask -> ACT
            # copy -> DVE/POOL sub) runs LAG pairs ahead of the aggregation
            # so the per-pair cross-engine latency never stalls the PE.
            LAG = 3
            for k in range(NP + LAG):
                if k < NP:
                    front(k)
                if k >= LAG:
                    agg(k - LAG)

            out_r = out.rearrange("(ic p) d -> ic p d", p=128)
            for ic in range(IC):
                ot = osb.tile([128, D], F16, tag="ot", name=f"ot{ic}")
                nc.scalar.copy(ot[:], psO[ic][:])
                (nc.sync if ic % 2 == 0 else nc.scalar).dma_start(
                    out=out_r[ic], in_=ot[:]
                )


_CACHE = {}


def _build1():
    if "p1" in _CACHE:
        return _CACHE["p1"]
    nc = bacc.Bacc("TRN2", target_bir_lowering=False, debug=False,
                   num_devices=NCORES)
    hst_in = nc.dram_tensor("hst_in", [128, IC * DC * 128], F16,
                            kind="ExternalInput").ap()
    p_in = nc.dram_tensor("p_in", [128, DC * H], F16, kind="ExternalInput").ap()
    w_out = nc.dram_tensor("w_out", [128, IC * H], F16,
                           kind="ExternalOutput").ap()
    with tile.TileContext(nc) as tc:
        _body1(tc, hst_in, p_in, w_out)
    nc.compile()
    _CACHE["p1"] = nc
    return nc


def _build2():
    if "p2" in _CACHE:
        return _CACHE["p2"]
    nc = bacc.Bacc("TRN2", target_bir_lowering=False, debug=False,
                   num_devices=NCORES)
    a8_in = nc.dram_tensor("a8_in", [128, JC * SH], F8,
                           kind="ExternalInput").ap()
    hh_in = nc.dram_tensor("hh_in", [128, JC * D], F8,
                           kind="ExternalInput").ap()
    hl_in = nc.dram_tensor("hl_in", [128, JC * D], F8,
                           kind="ExternalInput").ap()
    wt_in = nc.dram_tensor("wt_in", [3, N], F16, kind="ExternalInput").ap()
    w4_in = nc.dram_tensor("w4_in", [128, JC * 4], F16,
                           kind="ExternalInput").ap()
    id_in = nc.dram_tensor("id_in", [128, 128], F16, kind="ExternalInput").ap()
    out = nc.dram_tensor("out", [SH, D], F16, kind="ExternalOutput").ap()
    with tile.TileContext(nc) as tc:
        _body2(tc, a8_in, hh_in, hl_in, wt_in, w4_in, id_in, out)
    nc.compile()
    _CACHE["p2"] = nc
    return nc


def kernel(graph_info, h, P, _trace=False, _results_out=None):
    graph_info = np.ascontiguousarray(graph_info, dtype=np.float32)
    h = np.ascontiguousarray(h, dtype=np.float32)
    P = np.ascontiguousarray(P, dtype=np.float32)
    nc1 = _build1()
    nc2 = _build2()

    # host-side shard/layout prep (pure data movement + dtype casts)
    h16_full = h.astype(np.float16)
    p16_host = np.ascontiguousarray(
        P.astype(np.float16).reshape(DC, 128, H).transpose(1, 0, 2)
    ).reshape(128, DC * H)
    in1 = []
    for c in range(NCORES):
        hsT = h16_full[c * SH:(c + 1) * SH, :].T  # [D, SH]
        hst_host = np.ascontiguousarray(
            hsT.reshape(DC, 128, IC, 128).transpose(1, 2, 0, 3)
        ).reshape(128, IC * DC * 128)
        in1.append({"hst_in": hst_host, "p_in": p16_host})
    res1 = bass_utils.run_bass_kernel_spmd(
        nc1, in1, core_ids=list(range(NCORES)), trace=_trace
    )
    w_full = np.concatenate(
        [
            res1.results[c]["w_out"]
            .reshape(128, IC, H).transpose(1, 0, 2).reshape(SH, H)
            for c in range(NCORES)
        ],
        axis=0,
    )  # [N, 3] fp16, scaled by 2^-4

    wt_host = np.ascontiguousarray(w_full.T)  # [3, N]
    w4_host = np.ascontiguousarray(
        np.concatenate(
            [w_full.reshape(JC, 128, H).transpose(1, 0, 2),
             np.ones((128, JC, 1), np.float16)],
            axis=2,
        ).reshape(128, JC * 4)
    )
    id_host = np.eye(128, dtype=np.float16)

    # fp8 hi/lo split of h (host-side re-encoding; h = hh + hl up to e4m3^2)
    h_hi = np.clip(h, -240, 240).astype(NP_F8)
    h_lo = (h - h_hi.astype(np.float32)).astype(NP_F8)
    hh_host = np.ascontiguousarray(
        h_hi.reshape(JC, 128, D).transpose(1, 0, 2)).reshape(128, JC * D)
    hl_host = np.ascontiguousarray(
        h_lo.reshape(JC, 128, D).transpose(1, 0, 2)).reshape(128, JC * D)

    in2 = []
    for c in range(NCORES):
        at = np.ascontiguousarray(
            graph_info[c * SH:(c + 1) * SH, :].T
        ).astype(NP_F8)                      # [N(j), SH(i)]
        a8_host = np.ascontiguousarray(
            at.reshape(JC, 128, SH).transpose(1, 0, 2)).reshape(128, JC * SH)
        in2.append({
            "a8_in": a8_host,
            "hh_in": hh_host,
            "hl_in": hl_host,
            "wt_in": wt_host,
            "w4_in": w4_host,
            "id_in": id_host,
        })
    res2 = bass_utils.run_bass_kernel_spmd(
        nc2, in2, core_ids=list(range(NCORES)), trace=_trace
    )
    if _results_out is not None:
        _results_out.extend([res1, res2])
    return np.concatenate(
        [res2.results[c]["out"].astype(np.float32) for c in range(NCORES)],
        axis=0,
    )


# revision 15
# speedup vs baseline: 1.1664x; 1.0322x over previous
"""GAT-style attention (gnn_message_passing) Trainium2 kernel, 8-core row-parallel.

Math (algebraically identical to the reference masked-softmax attention):
  E = relu(h @ P)                 [N,3]
  W' = max(exp(E - 4ln2), 1/16)   (= exp(relu(E))/16, fp16-safe range)
  denom'[i,k] = sum_j A[i,j] W'[j,k]   (k=3 slot sums ones -> rowsum[i])
  R'[i,k] = rowsum[i] / denom'[i,k]
  ct[j,i]  = sum_k W'[j,k] R'[i,k] = rowsum[i] * C[i,j]
  mt[j,i]  = A[i,j] * ct[j,i]
  out[i,:] = sum_j mt[j,i] h[j,:]

Two SPMD programs (cost-modeled collectives are ~15us fixed -> too slow; the
tiny [4096,3] W matrix crosses cores via a host gather between programs):
  P1 (per core): W'-shard [512,3] from host-transposed h-shard (fp16 — fp8
      h.T fails the error budget through the exponential).
  host: concat W'-shards; build wt [3,N] / w4 (W'|ones) layouts; cast
      A-shard.T to fp8 (binary, exact); split h into fp8 hi/lo halves
      (h = h_hi + h_lo, each e4m3; lossless-ish re-encoding).
  P2 (per core): denominators stream with the A.T pieces (at8-stationary
      matmuls, one PSUM accumulation "super-group"), R' chain, then 16
      jc-pair sweeps:
        ct pair (fp16 matmuls, [128,1024] PSUM)
        mt16 = at8 * ct          (DVE, the only full-size PSUM touch)
        mt_hi8 = fp8(mt16)       (ACT copies, a couple on POOL for balance)
        mt_lo8 = mt16 - mt_hi8   (DVE/POOL split)
        psO[ic] += DoubleRow fp8 matmuls: mt_hi.T@h_hi + mt_hi.T@h_lo
                   + mt_lo.T@h_hi   (3-term split => 0.3% rel err, 4x
                   cheaper than fp16 per the 0.5 cycles/row DR rate)
      Warm-up matmuls during the initial load defeat the PE clock ramp.
"""

import numpy as np
import ml_dtypes

import concourse.bass as bass
import concourse.mybir as mybir
import concourse.tile as tile
from concourse import bacc
from concourse import bass_utils

N = 4096
D = 512
H = 3
NCORES = 8
SH = N // NCORES          # 512 output rows per core
JC = N // 128             # 32 j-chunks
IC = SH // 128            # 4 i-chunks
DC = D // 128             # 4 d-chunks
NP = JC // 2              # 16 jc-pairs
F8 = mybir.dt.float8e4
F16 = mybir.dt.float16
F32 = mybir.dt.float32
LN2x4 = float(4.0 * np.log(2.0))   # W scaled by 2^-4 to stay in fp16 range
NP_F8 = ml_dtypes.float8_e4m3
DR = mybir.MatmulPerfMode.DoubleRow


def _body1(tc, hst_in, p_in, w_out):
    """P1: W'-shard [SH,3] from hst [128, IC*DC*128] (h-shard.T, jc-major:
    hst[:, jc, dc, :] = h.T d-chunk dc for j-chunk jc), loaded in 2 pieces.
    The E matmuls use hst as the stationary operand (3-column streams)."""
    nc = tc.nc
    with (
        tc.tile_pool(name="sb1", bufs=1) as sb,
        tc.tile_pool(name="ps1", bufs=1, space="PSUM") as ps,
    ):
        hst = sb.tile([128, IC * DC * 128], F16, tag="hst")
        p16 = sb.tile([128, DC * H], F16, tag="p16")
        wsE = sb.tile([128, IC * H], F16, tag="wsE")
        ebias = sb.tile([128, 1], F32, tag="ebias")
        nc.vector.memset(ebias[:], -LN2x4)
        hst_v = hst[:].rearrange("p (g x) -> g p x", g=2)
        hin_v = hst_in.rearrange("p (g x) -> g p x", g=2)
        for g in range(2):
            nc.sync.dma_start(out=hst_v[g], in_=hin_v[g])
        nc.gpsimd.dma_start(out=p16[:], in_=p_in)

        # one PSUM tile spanning 4 banks: E group per jc, single exp at the end
        psE = ps.tile([128, IC * 512], F32, tag="psE", name="psE")
        for jc in range(IC):
            for dc in range(DC):
                nc.tensor.matmul(
                    psE[:, jc * 512: jc * 512 + H],
                    hst[:, (jc * DC + dc) * 128: (jc * DC + dc + 1) * 128],
                    p16[:, dc * H:(dc + 1) * H],
                    start=(dc == 0),
                    stop=(dc == DC - 1),
                )
        nc.scalar.activation(
            wsE[:].rearrange("p (jc k) -> p jc k", k=H),
            psE[:].rearrange("p (jc x) -> p jc x", x=512)[:, :, 0:H],
            mybir.ActivationFunctionType.Exp,
            bias=ebias[:], scale=1.0,
        )
        nc.vector.tensor_scalar_max(wsE[:], wsE[:], 0.0625)
        nc.sync.dma_start(out=w_out, in_=wsE[:])


def _body2(tc, a8_in, hh_in, hl_in, wt_in, w4_in, id_in, out):
    """P2: denominators + R' chain + 16 jc-pair sweeps with 3-term fp8
    DoubleRow aggregation. a8_in is A-shard.T fp8 [p, jc, i] packed;
    hh_in/hl_in are the fp8 hi/lo splits of h in [p, jc, d] packed order."""
    nc = tc.nc
    mult = mybir.AluOpType.mult
    subop = mybir.AluOpType.subtract

    with (
        tc.tile_pool(name="big", bufs=1) as big,
        tc.tile_pool(name="small", bufs=1) as small,
        tc.tile_pool(name="mtp", bufs=5) as mtp,
        tc.tile_pool(name="osb", bufs=4) as osb,
    ):
        at8 = big.tile([128, JC * SH], F8, tag="at8")       # A.T [p, jc, i]
        hh8 = big.tile([128, JC * D], F8, tag="hh8")        # h hi [p, jc, d]
        hl8 = big.tile([128, JC * D], F8, tag="hl8")        # h lo [p, jc, d]
        wt = small.tile([3, N], F16, tag="wt")              # W'.T
        w4 = small.tile([128, JC * 4], F16, tag="w4")       # W'|ones (j part)
        id16 = small.tile([128, 128], F16, tag="id16")
        scr = small.tile([128, 512], F16, tag="scr")        # warm-up source
        rN = small.tile([128, IC * H], F32, tag="rN")
        rN16 = small.tile([128, IC * H], F16, tag="rN16")   # rowsum/denom'
        rT16 = small.tile([3, SH], F16, tag="rT16")         # R'.T [k, i]

        # ---------------- loads ----------------
        # sync/HWDGE queue: A.T pieces first (denominators stream with them),
        # then h hi/lo pair-major pieces. Tiny tensors go on the scalar
        # HWDGE queue so they don't delay the at8 stream's SP-SEQ issue.
        nc.scalar.dma_start(out=w4[:], in_=w4_in)
        nc.scalar.dma_start(out=id16[:], in_=id_in)
        nc.scalar.dma_start(out=wt[:], in_=wt_in)
        NPIECE = 4
        PA = JC // NPIECE                                   # 8 jc per piece
        for pc in range(NPIECE):
            s = pc * PA * SH
            nc.sync.dma_start(out=at8[:, s:s + PA * SH],
                              in_=a8_in[:, s:s + PA * SH])
        for pc in range(4):
            s = pc * (JC // 4) * D
            nc.sync.dma_start(out=hh8[:, s:s + (JC // 4) * D],
                              in_=hh_in[:, s:s + (JC // 4) * D])
            nc.sync.dma_start(out=hl8[:, s:s + (JC // 4) * D],
                              in_=hl_in[:, s:s + (JC // 4) * D])

        nc.vector.memset(scr[:], 0.0)
        # warm the ACT table (LoadActFuncSet) off the critical path
        actw = small.tile([1, 2], F16, tag="actw")
        nc.scalar.copy(actw[:], scr[0:1, 0:2])

        with tc.tile_pool(name="pse", bufs=1, space="PSUM") as pse:
            # early PSUM pool: warm-up targets + denominators + R transposes;
            # closed before the sweep pools open so the banks are reused.
            psD2 = pse.tile([128, IC * 4], F32, tag="psd", name="psD2")
            psRT = pse.tile([3, SH], F16, tag="psrt", name="psRT")
            n_warm = 0

            def warm(n):
                nonlocal n_warm
                for _ in range(n):
                    pw = pse.tile([128, 512], F32, tag="warm",
                                  name=f"warm{n_warm}")
                    nc.tensor.matmul(
                        pw[:], scr[:, 0:128], scr[:], start=True, stop=True
                    )
                    n_warm += 1

            # psD2[p_i, ic*4+k] = sum_j A[i,j] W'[j,k]; k=3 gives rowsum.
            # One accumulation super-group: start only on the very first
            # matmul (pending-zero covers the whole bank region). Denominator
            # matmuls stream with the at8 pieces; warm-up matmuls fill the
            # PE gaps between pieces so the clock ramp isn't reset.
            warm(2)
            first = True
            for pc in range(NPIECE):
                for jc in range(pc * PA, (pc + 1) * PA):
                    for ic in range(IC):
                        nc.tensor.matmul(
                            psD2[:, ic * 4:(ic + 1) * 4],
                            at8[:, jc * SH + ic * 128:
                                jc * SH + ic * 128 + 128],
                            w4[:, jc * 4:(jc + 1) * 4],
                            start=first,
                            stop=(jc == JC - 1 and ic == IC - 1),
                            skip_group_check=True,
                        )
                        first = False
                if pc < NPIECE - 1:
                    warm(2)

            # R' = rowsum * 1/denom', transposed to [k, i]
            psD2_v = psD2[:].rearrange("p (ic s) -> p ic s", s=4)
            nc.vector.reciprocal(
                rN[:].rearrange("p (ic k) -> p ic k", k=H),
                psD2_v[:, :, 0:H],
            )
            for ic in range(IC):
                nc.vector.tensor_scalar(
                    rN16[:, ic * H:(ic + 1) * H], rN[:, ic * H:(ic + 1) * H],
                    psD2[:, ic * 4 + 3: ic * 4 + 4], None, op0=mult,
                )
                nc.tensor.transpose(
                    psRT[:, ic * 128:(ic + 1) * 128],
                    rN16[:, ic * H:(ic + 1) * H],
                    id16[:],
                )
            nc.vector.tensor_copy(rT16[:], psRT[:])

        with (
            tc.tile_pool(name="psc", bufs=2, space="PSUM") as psc,
            tc.tile_pool(name="pso", bufs=1, space="PSUM") as pso,
        ):
            psO = [
                pso.tile([128, D], F32, tag=f"psO{ic}", name=f"psO{ic}")
                for ic in range(IC)
            ]
            hh8_v = hh8[:].rearrange("p (pr two d) -> p pr two d", two=2, d=D)
            hl8_v = hl8[:].rearrange("p (pr two d) -> p pr two d", two=2, d=D)

            # engine assignment per pair for the hi-copy and lo-sub: POOL
            # takes most subs (DVE is mask-bound), but the tail pairs use
            # DVE so the drain chain isn't gated by POOL's 2.1us subs.
            cp_eng = ["act"] * NP
            sub_eng = [
                "dve" if (k % 5 == 0 or k >= NP - 3) else "pool"
                for k in range(NP)
            ]

            his = {}
            los = {}

            def front(k):
                # ct pair -> mask -> hi copy -> lo sub for pair k
                ctp = psc.tile([128, 2 * SH], F32, tag="ctp", name=f"ctp{k}")
                for half in range(2):
                    jc = 2 * k + half
                    nc.tensor.matmul(
                        ctp[:, half * SH:(half + 1) * SH],
                        wt[0:3, jc * 128:(jc + 1) * 128],
                        rT16[:],
                        start=True, stop=True,
                        tile_position=(0, 0),
                    )
                mt16 = mtp.tile([128, 2 * SH], F16, tag="mt16",
                                name=f"mt16_{k}")
                nc.vector.tensor_tensor(
                    mt16[:], at8[:, 2 * k * SH:(2 * k + 2) * SH], ctp[:],
                    op=mult,
                )
                hi8 = mtp.tile([128, 2 * SH], F8, tag="hi8", name=f"hi8_{k}")
                lo8 = mtp.tile([128, 2 * SH], F8, tag="lo8", name=f"lo8_{k}")
                if cp_eng[k] == "act":
                    nc.scalar.copy(hi8[:], mt16[:])
                else:
                    nc.gpsimd.tensor_copy(hi8[:], mt16[:])
                if sub_eng[k] == "dve":
                    nc.vector.tensor_tensor(lo8[:], mt16[:], hi8[:], op=subop)
                else:
                    nc.gpsimd.tensor_tensor(lo8[:], mt16[:], hi8[:], op=subop)
                his[k], los[k] = hi8, lo8

            def agg(k):
                hi8_v = his[k][:].rearrange("p (two i) -> p two i", two=2)
                lo8_v = los[k][:].rearrange("p (two i) -> p two i", two=2)
                for term, (lt, rt) in enumerate(
                    ((hi8_v, hh8_v), (hi8_v, hl8_v), (lo8_v, hh8_v))
                ):
                    for ic in range(IC):
                        nc.tensor.matmul(
                            psO[ic][:],
                            lt[:, :, ic * 128:(ic + 1) * 128],
                            rt[:, k],
                            start=(k == 0 and term == 0),
                            stop=(k == NP - 1 and term == 2),
                            perf_mode=DR,
                            skip_group_check=True,
                        )

            # software pipeline: the front chain (PE ct -> DVE mask -> ACT
            # copy -> DVE/POOL sub) runs LAG pairs ahead of the aggregation
            # so the per-pair cross-engine latency never stalls the PE.
            LAG = 3
            for k in range(NP + LAG):
                if k < NP:
                    front(k)
                if k >= LAG:
                    agg(k - LAG)

            out_r = out.rearrange("(ic p) d -> ic p d", p=128)
            for ic in range(IC):
                ot = osb.tile([128, D], F16, tag="ot", name=f"ot{ic}")
                if ic % 2 == 0:
                    nc.scalar.copy(ot[:], psO[ic][:])
                else:
                    nc.vector.tensor_copy(ot[:], psO[ic][:])
                (nc.sync if ic % 2 == 0 else nc.scalar).dma_start(
                    out=out_r[ic], in_=ot[:]
                )


_CACHE = {}


def _build1():
    if "p1" in _CACHE:
        return _CACHE["p1"]
    nc = bacc.Bacc("TRN2", target_bir_lowering=False, debug=False,
                   num_devices=NCORES)
    hst_in = nc.dram_tensor("hst_in", [128, IC * DC * 128], F16,
                            kind="ExternalInput").ap()
    p_in = nc.dram_tensor("p_in", [128, DC * H], F16, kind="ExternalInput").ap()
    w_out = nc.dram_tensor("w_out", [128, IC * H], F16,
                           kind="ExternalOutput").ap()
    with tile.TileContext(nc) as tc:
        _body1(tc, hst_in, p_in, w_out)
    nc.compile()
    _CACHE["p1"] = nc
    return nc


def _build2():
    if "p2" in _CACHE:
        return _CACHE["p2"]
    nc = bacc.Bacc("TRN2", target_bir_lowering=False, debug=False,
                   num_devices=NCORES)
    a8_in = nc.dram_tensor("a8_in", [128, JC * SH], F8,
                           kind="ExternalInput").ap()
    hh_in = nc.dram_tensor("hh_in", [128, JC * D], F8,
                           kind="ExternalInput").ap()
    hl_in = nc.dram_tensor("hl_in", [128, JC * D], F8,
                           kind="ExternalInput").ap()
    wt_in = nc.dram_tensor("wt_in", [3, N], F16, kind="ExternalInput").ap()
    w4_in = nc.dram_tensor("w4_in", [128, JC * 4], F16,
                           kind="ExternalInput").ap()
    id_in = nc.dram_tensor("id_in", [128, 128], F16, kind="ExternalInput").ap()
    out = nc.dram_tensor("out", [SH, D], F16, kind="ExternalOutput").ap()
    with tile.TileContext(nc) as tc:
        _body2(tc, a8_in, hh_in, hl_in, wt_in, w4_in, id_in, out)
    nc.compile()
    _CACHE["p2"] = nc
    return nc


def kernel(graph_info, h, P, _trace=False, _results_out=None):
    graph_info = np.ascontiguousarray(graph_info, dtype=np.float32)
    h = np.ascontiguousarray(h, dtype=np.float32)
    P = np.ascontiguousarray(P, dtype=np.float32)
    nc1 = _build1()
    nc2 = _build2()

    # host-side shard/layout prep (pure data movement + dtype casts)
    h16_full = h.astype(np.float16)
    p16_host = np.ascontiguousarray(
        P.astype(np.float16).reshape(DC, 128, H).transpose(1, 0, 2)
    ).reshape(128, DC * H)
    in1 = []
    for c in range(NCORES):
        hsT = h16_full[c * SH:(c + 1) * SH, :].T  # [D, SH]
        hst_host = np.ascontiguousarray(
            hsT.reshape(DC, 128, IC, 128).transpose(1, 2, 0, 3)
        ).reshape(128, IC * DC * 128)
        in1.append({"hst_in": hst_host, "p_in": p16_host})
    res1 = bass_utils.run_bass_kernel_spmd(
        nc1, in1, core_ids=list(range(NCORES)), trace=_trace
    )
    w_full = np.concatenate(
        [
            res1.results[c]["w_out"]
            .reshape(128, IC, H).transpose(1, 0, 2).reshape(SH, H)
            for c in range(NCORES)
        ],
        axis=0,
    )  # [N, 3] fp16, scaled by 2^-4

    wt_host = np.ascontiguousarray(w_full.T)  # [3, N]
    w4_host = np.ascontiguousarray(
        np.concatenate(
            [w_full.reshape(JC, 128, H).transpose(1, 0, 2),
             np.ones((128, JC, 1), np.float16)],
            axis=2,
        ).reshape(128, JC * 4)
    )
    id_host = np.eye(128, dtype=np.float16)

    # fp8 hi/lo split of h (host-side re-encoding; h = hh + hl up to e4m3^2)
    h_hi = np.clip(h, -240, 240).astype(NP_F8)
    h_lo = (h - h_hi.astype(np.float32)).astype(NP_F8)
    hh_host = np.ascontiguousarray(
        h_hi.reshape(JC, 128, D).transpose(1, 0, 2)).reshape(128, JC * D)
    hl_host = np.ascontiguousarray(
        h_lo.reshape(JC, 128, D).transpose(1, 0, 2)).reshape(128, JC * D)

    in2 = []
    for c in range(NCORES):
        at = np.ascontiguousarray(
            graph_info[c * SH:(c + 1) * SH, :].T
        ).astype(NP_F8)                      # [N(j), SH(i)]
        a8_host = np.ascontiguousarray(
            at.reshape(JC, 128, SH).transpose(1, 0, 2)).reshape(128, JC * SH)
        in2.append({
            "a8_in": a8_host,
            "hh_in": hh_host,
            "hl_in": hl_host,
            "wt_in": wt_host,
            "w4_in": w4_host,
            "id_in": id_host,
        })
    res2 = bass_utils.run_bass_kernel_spmd(
        nc2, in2, core_ids=list(range(NCORES)), trace=_trace
    )
    if _results_out is not None:
        _results_out.extend([res1, res2])
    return np.concatenate(
        [res2.results[c]["out"].astype(np.float32) for c in range(NCORES)],
        axis=0,
    )


# revision 22
# speedup vs baseline: 1.2018x; 1.0303x over previous
"""GAT-style attention (gnn_message_passing) Trainium2 kernel, 8-core row-parallel.

Math (algebraically identical to the reference masked-softmax attention):
  E = relu(h @ P)                 [N,3]
  W' = max(exp(E - 4ln2), 1/16)   (= exp(relu(E))/16, fp16-safe range)
  denom'[i,k] = sum_j A[i,j] W'[j,k]   (k=3 slot sums ones -> rowsum[i])
  R'[i,k] = rowsum[i] / denom'[i,k]
  ct[j,i]  = sum_k W'[j,k] R'[i,k] = rowsum[i] * C[i,j]
  mt[j,i]  = A[i,j] * ct[j,i]
  out[i,:] = sum_j mt[j,i] h[j,:]

Two SPMD programs (cost-modeled collectives are ~15us fixed -> too slow; the
tiny [4096,3] W matrix crosses cores via a host gather between programs):
  P1 (per core): W'-shard [512,3] from host-transposed h-shard (fp16 — fp8
      h.T fails the error budget through the exponential).
  host: concat W'-shards; build wt [3,N] / w4 (W'|ones) layouts; cast
      A-shard.T to fp8 (binary, exact); split h into fp8 hi/lo halves
      (h = h_hi + h_lo, each e4m3; lossless-ish re-encoding).
  P2 (per core): denominators stream with the A.T pieces (at8-stationary
      matmuls, one PSUM accumulation "super-group"), R' chain, then 16
      jc-pair sweeps:
        ct pair (fp16 matmuls, [128,1024] PSUM)
        mt16 = at8 * ct          (DVE, the only full-size PSUM touch)
        mt_hi8 = fp8(mt16)       (ACT copies, a couple on POOL for balance)
        mt_lo8 = mt16 - mt_hi8   (DVE/POOL split)
        psO[ic] += DoubleRow fp8 matmuls: mt_hi.T@h_hi + mt_hi.T@h_lo
                   + mt_lo.T@h_hi   (3-term split => 0.3% rel err, 4x
                   cheaper than fp16 per the 0.5 cycles/row DR rate)
      Warm-up matmuls during the initial load defeat the PE clock ramp.
"""

import numpy as np
import ml_dtypes

import concourse.bass as bass
import concourse.mybir as mybir
import concourse.tile as tile
from concourse import bacc
from concourse import bass_utils

N = 4096
D = 512
H = 3
NCORES = 8
SH = N // NCORES          # 512 output rows per core
JC = N // 128             # 32 j-chunks
IC = SH // 128            # 4 i-chunks
DC = D // 128             # 4 d-chunks
NP = JC // 2              # 16 jc-pairs
F8 = mybir.dt.float8e4
F16 = mybir.dt.float16
F32 = mybir.dt.float32
LN2x4 = float(4.0 * np.log(2.0))   # W scaled by 2^-4 to stay in fp16 range
NP_F8 = ml_dtypes.float8_e4m3
DR = mybir.MatmulPerfMode.DoubleRow


def _body1(tc, hst_in, p_in, w_out):
    """P1: W'-shard [SH,3] from hst [128, IC*DC*128] (h-shard.T, jc-major:
    hst[:, jc, dc, :] = h.T d-chunk dc for j-chunk jc), loaded in 2 pieces.
    The E matmuls use hst as the stationary operand (3-column streams)."""
    nc = tc.nc
    with (
        tc.tile_pool(name="sb1", bufs=1) as sb,
        tc.tile_pool(name="ps1", bufs=1, space="PSUM") as ps,
    ):
        hst = sb.tile([128, IC * DC * 128], F16, tag="hst")
        p16 = sb.tile([128, DC * H], F16, tag="p16")
        wsE = sb.tile([128, IC * H], F16, tag="wsE")
        ebias = sb.tile([128, 1], F32, tag="ebias")
        nc.vector.memset(ebias[:], -LN2x4)
        hst_v = hst[:].rearrange("p (g x) -> g p x", g=2)
        hin_v = hst_in.rearrange("p (g x) -> g p x", g=2)
        for g in range(2):
            nc.sync.dma_start(out=hst_v[g], in_=hin_v[g])
        nc.gpsimd.dma_start(out=p16[:], in_=p_in)

        # one PSUM tile spanning 4 banks: E group per jc, single exp at the end
        psE = ps.tile([128, IC * 512], F32, tag="psE", name="psE")
        for jc in range(IC):
            for dc in range(DC):
                nc.tensor.matmul(
                    psE[:, jc * 512: jc * 512 + H],
                    hst[:, (jc * DC + dc) * 128: (jc * DC + dc + 1) * 128],
                    p16[:, dc * H:(dc + 1) * H],
                    start=(dc == 0),
                    stop=(dc == DC - 1),
                )
        nc.scalar.activation(
            wsE[:].rearrange("p (jc k) -> p jc k", k=H),
            psE[:].rearrange("p (jc x) -> p jc x", x=512)[:, :, 0:H],
            mybir.ActivationFunctionType.Exp,
            bias=ebias[:], scale=1.0,
        )
        nc.vector.tensor_scalar_max(wsE[:], wsE[:], 0.0625)
        nc.sync.dma_start(out=w_out, in_=wsE[:])


def _body2(tc, a8_in, hh_in, hl_in, wt_in, w4_in, id_in, out):
    """P2: denominators + R' chain + 16 jc-pair sweeps with 3-term fp8
    DoubleRow aggregation. a8_in is A-shard.T fp8 [p, jc, i] packed;
    hh_in/hl_in are the fp8 hi/lo splits of h in [p, jc, d] packed order."""
    nc = tc.nc
    mult = mybir.AluOpType.mult
    subop = mybir.AluOpType.subtract

    with (
        tc.tile_pool(name="big", bufs=1) as big,
        tc.tile_pool(name="small", bufs=1) as small,
        tc.tile_pool(name="mtp", bufs=6) as mtp,
        tc.tile_pool(name="osb", bufs=4) as osb,
    ):
        at8 = big.tile([128, JC * SH], F8, tag="at8")       # A.T [p, jc, i]
        hh8 = big.tile([128, JC * D], F8, tag="hh8")        # h hi [p, jc, d]
        hl8 = big.tile([128, JC * D], F8, tag="hl8")        # h lo [p, jc, d]
        wt = small.tile([3, N], F16, tag="wt")              # W'.T
        w4 = small.tile([128, JC * 4], F16, tag="w4")       # W'|ones (j part)
        id16 = small.tile([128, 128], F16, tag="id16")
        scr = small.tile([128, 512], F16, tag="scr")        # warm-up source
        rN16 = small.tile([128, IC * H], F16, tag="rN16")   # 1/denom'
        rs32 = small.tile([128, IC], F32, tag="rs32")       # rowsum per ic
        rT16 = small.tile([3, SH], F16, tag="rT16")         # R'.T [k, i]

        # ---------------- loads ----------------
        # sync/HWDGE queue: A.T pieces first (denominators stream with them),
        # then h hi/lo pair-major pieces. Tiny tensors go on the scalar
        # HWDGE queue so they don't delay the at8 stream's SP-SEQ issue.
        nc.scalar.dma_start(out=w4[:], in_=w4_in)
        nc.scalar.dma_start(out=id16[:], in_=id_in)
        nc.scalar.dma_start(out=wt[:], in_=wt_in)
        # at8 piece sizes in jc: big pieces first, a tiny last piece so the
        # final DMA-completion semaphore (+900ns) lands as early as possible.
        PIECES = [12, 12, 7, 1]
        off = 0
        bounds = []
        for n in PIECES:
            nc.sync.dma_start(out=at8[:, off * SH:(off + n) * SH],
                              in_=a8_in[:, off * SH:(off + n) * SH])
            bounds.append((off, off + n))
            off += n
        for pc in range(4):
            s = pc * (JC // 4) * D
            nc.sync.dma_start(out=hh8[:, s:s + (JC // 4) * D],
                              in_=hh_in[:, s:s + (JC // 4) * D])
            nc.sync.dma_start(out=hl8[:, s:s + (JC // 4) * D],
                              in_=hl_in[:, s:s + (JC // 4) * D])

        nc.vector.memset(scr[:], 0.0)
        # warm the ACT table (LoadActFuncSet) off the critical path
        actw = small.tile([1, 2], F16, tag="actw")
        nc.scalar.copy(actw[:], scr[0:1, 0:2])

        with tc.tile_pool(name="pse", bufs=1, space="PSUM") as pse:
            # early PSUM pool: warm-up targets + denominators + R transposes;
            # closed before the sweep pools open so the banks are reused.
            psD2 = pse.tile([128, IC * 4], F32, tag="psd", name="psD2")
            psRT = pse.tile([3, SH], F16, tag="psrt", name="psRT")
            n_warm = 0

            def warm(n):
                nonlocal n_warm
                for _ in range(n):
                    pw = pse.tile([128, 512], F32, tag="warm",
                                  name=f"warm{n_warm}")
                    nc.tensor.matmul(
                        pw[:], scr[:, 0:128], scr[:], start=True, stop=True
                    )
                    n_warm += 1

            # psD2[p_i, ic*4+k] = sum_j A[i,j] W'[j,k]; k=3 gives rowsum.
            # One accumulation super-group: start only on the very first
            # matmul (pending-zero covers the whole bank region). Denominator
            # matmuls stream with the at8 pieces; warm-up matmuls fill the
            # PE gaps between pieces so the clock ramp isn't reset.
            warm(4)
            first = True
            for pi, (j0, j1) in enumerate(bounds):
                for jc in range(j0, j1):
                    for ic in range(IC):
                        nc.tensor.matmul(
                            psD2[:, ic * 4:(ic + 1) * 4],
                            at8[:, jc * SH + ic * 128:
                                jc * SH + ic * 128 + 128],
                            w4[:, jc * 4:(jc + 1) * 4],
                            start=first,
                            stop=(jc == JC - 1 and ic == IC - 1),
                            skip_group_check=True,
                        )
                        first = False
                if pi < len(bounds) - 2:
                    warm(3)

            # R' = 1/denom' (fp16), transposed to [k, i]. The rowsum factor
            # is folded into the final psO stores (per-partition scale).
            psD2_v = psD2[:].rearrange("p (ic s) -> p ic s", s=4)
            with nc.allow_low_precision(reason="R' fits fp16 comfortably"):
                nc.vector.reciprocal(
                    rN16[:].rearrange("p (ic k) -> p ic k", k=H),
                    psD2_v[:, :, 0:H],
                )
            nc.vector.tensor_copy(rs32[:], psD2_v[:, :, 3])
            for ic in range(IC):
                nc.tensor.transpose(
                    psRT[:, ic * 128:(ic + 1) * 128],
                    rN16[:, ic * H:(ic + 1) * H],
                    id16[:],
                )
            nc.vector.tensor_copy(rT16[:], psRT[:])

        with (
            tc.tile_pool(name="psc", bufs=2, space="PSUM") as psc,
            tc.tile_pool(name="pso", bufs=1, space="PSUM") as pso,
        ):
            psO = [
                pso.tile([128, D], F32, tag=f"psO{ic}", name=f"psO{ic}")
                for ic in range(IC)
            ]
            hh8_v = hh8[:].rearrange("p (pr two d) -> p pr two d", two=2, d=D)
            hl8_v = hl8[:].rearrange("p (pr two d) -> p pr two d", two=2, d=D)

            # engine assignment per pair for the hi-copy and lo-sub: POOL
            # takes most subs (DVE is mask-bound), but the tail pairs use
            # DVE so the drain chain isn't gated by POOL's 2.1us subs.
            cp_eng = ["act"] * NP
            sub_eng = [
                "dve" if (k % 5 == 0 or k >= NP - 3) else "pool"
                for k in range(NP)
            ]

            his = {}
            los = {}

            def front(k):
                # ct pair -> mask -> hi copy -> lo sub for pair k
                ctp = psc.tile([128, 2 * SH], F32, tag="ctp", name=f"ctp{k}")
                for half in range(2):
                    jc = 2 * k + half
                    nc.tensor.matmul(
                        ctp[:, half * SH:(half + 1) * SH],
                        wt[0:3, jc * 128:(jc + 1) * 128],
                        rT16[:],
                        start=True, stop=True,
                        tile_position=(0, 0),
                    )
                mt16 = mtp.tile([128, 2 * SH], F16, tag="mt16",
                                name=f"mt16_{k}")
                nc.vector.tensor_tensor(
                    mt16[:], at8[:, 2 * k * SH:(2 * k + 2) * SH], ctp[:],
                    op=mult,
                )
                hi8 = mtp.tile([128, 2 * SH], F8, tag="hi8", name=f"hi8_{k}")
                lo8 = mtp.tile([128, 2 * SH], F8, tag="lo8", name=f"lo8_{k}")
                if cp_eng[k] == "act":
                    nc.scalar.copy(hi8[:], mt16[:])
                else:
                    nc.gpsimd.tensor_copy(hi8[:], mt16[:])
                if sub_eng[k] == "dve":
                    nc.vector.tensor_tensor(lo8[:], mt16[:], hi8[:], op=subop)
                else:
                    nc.gpsimd.tensor_tensor(lo8[:], mt16[:], hi8[:], op=subop)
                his[k], los[k] = hi8, lo8

            out_r = out.rearrange("(ic p) d -> ic p d", p=128)

            def store(ic):
                # rowsum-scaled PSUM->SBUF copy + out DMA
                ot = osb.tile([128, D], F16, tag="ot", name=f"ot{ic}")
                if ic % 2 == 0:
                    nc.scalar.mul(ot[:], psO[ic][:], rs32[:, ic:ic + 1])
                else:
                    nc.vector.tensor_scalar(
                        ot[:], psO[ic][:], rs32[:, ic:ic + 1], None, op0=mult
                    )
                (nc.sync if ic % 2 == 0 else nc.scalar).dma_start(
                    out=out_r[ic], in_=ot[:]
                )

            TERMS = ((0, 0), (0, 1), (1, 0))  # (hi/lo, hh/hl)

            def agg(k):
                hi8_v = his[k][:].rearrange("p (two i) -> p two i", two=2)
                lo8_v = los[k][:].rearrange("p (two i) -> p two i", two=2)
                lts = (hi8_v, lo8_v)
                rts = (hh8_v, hl8_v)
                if k < NP - 1:
                    order = [(t, ic) for t in range(3) for ic in range(IC)]
                else:
                    # last pair: ic-major so each psO bank finishes (stop)
                    # as early as possible, with its store right behind
                    order = [(t, ic) for ic in range(IC) for t in range(3)]
                for t, ic in order:
                    lt, rt = lts[TERMS[t][0]], rts[TERMS[t][1]]
                    nc.tensor.matmul(
                        psO[ic][:],
                        lt[:, :, ic * 128:(ic + 1) * 128],
                        rt[:, k],
                        start=(k == 0 and t == 0),
                        stop=(k == NP - 1 and t == 2),
                        perf_mode=DR,
                        skip_group_check=True,
                    )
                    if k == NP - 1 and t == 2:
                        store(ic)

            # software pipeline: the front chain (PE ct -> DVE mask -> ACT
            # copy -> DVE/POOL sub) runs LAG pairs ahead of the aggregation
            # so the per-pair cross-engine latency never stalls the PE.
            LAG = 4
            for k in range(NP + LAG):
                if k < NP:
                    front(k)
                if k >= LAG:
                    agg(k - LAG)


_CACHE = {}


def _build1():
    if "p1" in _CACHE:
        return _CACHE["p1"]
    nc = bacc.Bacc("TRN2", target_bir_lowering=False, debug=False,
                   num_devices=NCORES)
    hst_in = nc.dram_tensor("hst_in", [128, IC * DC * 128], F16,
                            kind="ExternalInput").ap()
    p_in = nc.dram_tensor("p_in", [128, DC * H], F16, kind="ExternalInput").ap()
    w_out = nc.dram_tensor("w_out", [128, IC * H], F16,
                           kind="ExternalOutput").ap()
    with tile.TileContext(nc) as tc:
        _body1(tc, hst_in, p_in, w_out)
    nc.compile()
    _CACHE["p1"] = nc
    return nc


def _build2():
    if "p2" in _CACHE:
        return _CACHE["p2"]
    nc = bacc.Bacc("TRN2", target_bir_lowering=False, debug=False,
                   num_devices=NCORES)
    a8_in = nc.dram_tensor("a8_in", [128, JC * SH], F8,
                           kind="ExternalInput").ap()
    hh_in = nc.dram_tensor("hh_in", [128, JC * D], F8,
                           kind="ExternalInput").ap()
    hl_in = nc.dram_tensor("hl_in", [128, JC * D], F8,
                           kind="ExternalInput").ap()
    wt_in = nc.dram_tensor("wt_in", [3, N], F16, kind="ExternalInput").ap()
    w4_in = nc.dram_tensor("w4_in", [128, JC * 4], F16,
                           kind="ExternalInput").ap()
    id_in = nc.dram_tensor("id_in", [128, 128], F16, kind="ExternalInput").ap()
    out = nc.dram_tensor("out", [SH, D], F16, kind="ExternalOutput").ap()
    with tile.TileContext(nc) as tc:
        _body2(tc, a8_in, hh_in, hl_in, wt_in, w4_in, id_in, out)
    nc.compile()
    _CACHE["p2"] = nc
    return nc


def kernel(graph_info, h, P, _trace=False, _results_out=None):
    graph_info = np.ascontiguousarray(graph_info, dtype=np.float32)
    h = np.ascontiguousarray(h, dtype=np.float32)
    P = np.ascontiguousarray(P, dtype=np.float32)
    nc1 = _build1()
    nc2 = _build2()

    # host-side shard/layout prep (pure data movement + dtype casts)
    h16_full = h.astype(np.float16)
    p16_host = np.ascontiguousarray(
        P.astype(np.float16).reshape(DC, 128, H).transpose(1, 0, 2)
    ).reshape(128, DC * H)
    in1 = []
    for c in range(NCORES):
        hsT = h16_full[c * SH:(c + 1) * SH, :].T  # [D, SH]
        hst_host = np.ascontiguousarray(
            hsT.reshape(DC, 128, IC, 128).transpose(1, 2, 0, 3)
        ).reshape(128, IC * DC * 128)
        in1.append({"hst_in": hst_host, "p_in": p16_host})
    res1 = bass_utils.run_bass_kernel_spmd(
        nc1, in1, core_ids=list(range(NCORES)), trace=_trace
    )
    w_full = np.concatenate(
        [
            res1.results[c]["w_out"]
            .reshape(128, IC, H).transpose(1, 0, 2).reshape(SH, H)
            for c in range(NCORES)
        ],
        axis=0,
    )  # [N, 3] fp16, scaled by 2^-4

    wt_host = np.ascontiguousarray(w_full.T)  # [3, N]
    w4_host = np.ascontiguousarray(
        np.concatenate(
            [w_full.reshape(JC, 128, H).transpose(1, 0, 2),
             np.ones((128, JC, 1), np.float16)],
            axis=2,
        ).reshape(128, JC * 4)
    )
    id_host = np.eye(128, dtype=np.float16)

    # fp8 hi/lo split of h (host-side re-encoding; h = hh + hl up to e4m3^2)
    h_hi = np.clip(h, -240, 240).astype(NP_F8)
    h_lo = (h - h_hi.astype(np.float32)).astype(NP_F8)
    hh_host = np.ascontiguousarray(
        h_hi.reshape(JC, 128, D).transpose(1, 0, 2)).reshape(128, JC * D)
    hl_host = np.ascontiguousarray(
        h_lo.reshape(JC, 128, D).transpose(1, 0, 2)).reshape(128, JC * D)

    in2 = []
    for c in range(NCORES):
        at = np.ascontiguousarray(
            graph_info[c * SH:(c + 1) * SH, :].T
        ).astype(NP_F8)                      # [N(j), SH(i)]
        a8_host = np.ascontiguousarray(
            at.reshape(JC, 128, SH).transpose(1, 0, 2)).reshape(128, JC * SH)
        in2.append({
            "a8_in": a8_host,
            "hh_in": hh_host,
            "hl_in": hl_host,
            "wt_in": wt_host,
            "w4_in": w4_host,
            "id_in": id_host,
        })
    res2 = bass_utils.run_bass_kernel_spmd(
        nc2, in2, core_ids=list(range(NCORES)), trace=_trace
    )
    if _results_out is not None:
        _results_out.extend([res1, res2])
    return np.concatenate(
        [res2.results[c]["out"].astype(np.float32) for c in range(NCORES)],
        axis=0,
    )


# revision 24
# speedup vs baseline: 1.2073x; 1.0046x over previous
"""GAT-style attention (gnn_message_passing) Trainium2 kernel, 8-core row-parallel.

Math (algebraically identical to the reference masked-softmax attention):
  E = relu(h @ P)                 [N,3]
  W' = max(exp(E - 4ln2), 1/16)   (= exp(relu(E))/16, fp16-safe range)
  denom'[i,k] = sum_j A[i,j] W'[j,k]   (k=3 slot sums ones -> rowsum[i])
  R'[i,k] = rowsum[i] / denom'[i,k]
  ct[j,i]  = sum_k W'[j,k] R'[i,k] = rowsum[i] * C[i,j]
  mt[j,i]  = A[i,j] * ct[j,i]
  out[i,:] = sum_j mt[j,i] h[j,:]

Two SPMD programs (cost-modeled collectives are ~15us fixed -> too slow; the
tiny [4096,3] W matrix crosses cores via a host gather between programs):
  P1 (per core): W'-shard [512,3] from host-transposed h-shard (fp16 — fp8
      h.T fails the error budget through the exponential).
  host: concat W'-shards; build wt [3,N] / w4 (W'|ones) layouts; cast
      A-shard.T to fp8 (binary, exact); split h into fp8 hi/lo halves
      (h = h_hi + h_lo, each e4m3; lossless-ish re-encoding).
  P2 (per core): denominators stream with the A.T pieces (at8-stationary
      matmuls, one PSUM accumulation "super-group"), R' chain, then 16
      jc-pair sweeps:
        ct pair (fp16 matmuls, [128,1024] PSUM)
        mt16 = at8 * ct          (DVE, the only full-size PSUM touch)
        mt_hi8 = fp8(mt16)       (ACT copies, a couple on POOL for balance)
        mt_lo8 = mt16 - mt_hi8   (DVE/POOL split)
        psO[ic] += DoubleRow fp8 matmuls: mt_hi.T@h_hi + mt_hi.T@h_lo
                   + mt_lo.T@h_hi   (3-term split => 0.3% rel err, 4x
                   cheaper than fp16 per the 0.5 cycles/row DR rate)
      Warm-up matmuls during the initial load defeat the PE clock ramp.
"""

import numpy as np
import ml_dtypes

import concourse.bass as bass
import concourse.mybir as mybir
import concourse.tile as tile
from concourse import bacc
from concourse import bass_utils

N = 4096
D = 512
H = 3
NCORES = 8
SH = N // NCORES          # 512 output rows per core
JC = N // 128             # 32 j-chunks
IC = SH // 128            # 4 i-chunks
DC = D // 128             # 4 d-chunks
NP = JC // 2              # 16 jc-pairs
F8 = mybir.dt.float8e4
F16 = mybir.dt.float16
F32 = mybir.dt.float32
LN2x4 = float(4.0 * np.log(2.0))   # W scaled by 2^-4 to stay in fp16 range
NP_F8 = ml_dtypes.float8_e4m3
DR = mybir.MatmulPerfMode.DoubleRow


def _body1(tc, hst_in, p_in, w_out):
    """P1: W'-shard [SH,3] from hst [128, IC*DC*128] (h-shard.T, jc-major:
    hst[:, jc, dc, :] = h.T d-chunk dc for j-chunk jc), loaded in 2 pieces.
    The E matmuls use hst as the stationary operand (3-column streams)."""
    nc = tc.nc
    with (
        tc.tile_pool(name="sb1", bufs=1) as sb,
        tc.tile_pool(name="ps1", bufs=1, space="PSUM") as ps,
    ):
        hst = sb.tile([128, IC * DC * 128], F16, tag="hst")
        p16 = sb.tile([128, DC * H], F16, tag="p16")
        wsE = sb.tile([128, IC * H], F16, tag="wsE")
        ebias = sb.tile([128, 1], F32, tag="ebias")
        nc.vector.memset(ebias[:], -LN2x4)
        hst_v = hst[:].rearrange("p (g x) -> g p x", g=2)
        hin_v = hst_in.rearrange("p (g x) -> g p x", g=2)
        for g in range(2):
            nc.sync.dma_start(out=hst_v[g], in_=hin_v[g])
        nc.gpsimd.dma_start(out=p16[:], in_=p_in)

        # one PSUM tile spanning 4 banks: E group per jc, single exp at the end
        psE = ps.tile([128, IC * 512], F32, tag="psE", name="psE")
        for jc in range(IC):
            for dc in range(DC):
                nc.tensor.matmul(
                    psE[:, jc * 512: jc * 512 + H],
                    hst[:, (jc * DC + dc) * 128: (jc * DC + dc + 1) * 128],
                    p16[:, dc * H:(dc + 1) * H],
                    start=(dc == 0),
                    stop=(dc == DC - 1),
                )
        nc.scalar.activation(
            wsE[:].rearrange("p (jc k) -> p jc k", k=H),
            psE[:].rearrange("p (jc x) -> p jc x", x=512)[:, :, 0:H],
            mybir.ActivationFunctionType.Exp,
            bias=ebias[:], scale=1.0,
        )
        nc.vector.tensor_scalar_max(wsE[:], wsE[:], 0.0625)
        nc.sync.dma_start(out=w_out, in_=wsE[:])


def _body2(tc, a8_in, hh_in, hl_in, wt_in, w4_in, id_in, out):
    """P2: denominators + R' chain + 16 jc-pair sweeps with 3-term fp8
    DoubleRow aggregation. a8_in is A-shard.T fp8 [p, jc, i] packed;
    hh_in/hl_in are the fp8 hi/lo splits of h in [p, jc, d] packed order."""
    nc = tc.nc
    mult = mybir.AluOpType.mult
    subop = mybir.AluOpType.subtract

    with (
        tc.tile_pool(name="big", bufs=1) as big,
        tc.tile_pool(name="small", bufs=1) as small,
        tc.tile_pool(name="mtp", bufs=6) as mtp,
        tc.tile_pool(name="osb", bufs=4) as osb,
    ):
        at8 = big.tile([128, JC * SH], F8, tag="at8")       # A.T [p, jc, i]
        hh8 = big.tile([128, JC * D], F8, tag="hh8")        # h hi [p, jc, d]
        hl8 = big.tile([128, JC * D], F8, tag="hl8")        # h lo [p, jc, d]
        wt = small.tile([3, N], F16, tag="wt")              # W'.T
        w4 = small.tile([128, JC * 4], F16, tag="w4")       # W'|ones (j part)
        id16 = small.tile([128, 128], F16, tag="id16")
        scr = small.tile([128, 512], F16, tag="scr")        # warm-up source
        rN16 = small.tile([128, IC * H], F16, tag="rN16")   # 1/denom'
        rs32 = small.tile([128, IC], F32, tag="rs32")       # rowsum per ic
        rT16 = small.tile([3, SH], F16, tag="rT16")         # R'.T [k, i]

        # ---------------- loads ----------------
        # sync/HWDGE queue: w4 (denominators need it early), A.T pieces
        # (denominators stream with them), id/wt (needed only at the R
        # transposes / first ct), then h hi/lo pair-major pieces.
        nc.scalar.dma_start(out=w4[:], in_=w4_in)
        # at8 piece sizes in jc: big pieces first, a tiny last piece so the
        # final DMA-completion semaphore (+900ns) lands as early as possible.
        PIECES = [12, 12, 7, 1]
        off = 0
        bounds = []
        for n in PIECES:
            nc.sync.dma_start(out=at8[:, off * SH:(off + n) * SH],
                              in_=a8_in[:, off * SH:(off + n) * SH])
            bounds.append((off, off + n))
            off += n
        nc.sync.dma_start(out=id16[:], in_=id_in)
        nc.sync.dma_start(out=wt[:], in_=wt_in)
        for pc in range(4):
            s = pc * (JC // 4) * D
            nc.sync.dma_start(out=hh8[:, s:s + (JC // 4) * D],
                              in_=hh_in[:, s:s + (JC // 4) * D])
            nc.sync.dma_start(out=hl8[:, s:s + (JC // 4) * D],
                              in_=hl_in[:, s:s + (JC // 4) * D])

        nc.vector.memset(scr[:], 0.0)
        # warm the ACT table (LoadActFuncSet) off the critical path
        actw = small.tile([1, 2], F16, tag="actw")
        nc.scalar.copy(actw[:], scr[0:1, 0:2])

        with tc.tile_pool(name="pse", bufs=1, space="PSUM") as pse:
            # early PSUM pool: warm-up targets + denominators + R transposes;
            # closed before the sweep pools open so the banks are reused.
            psD2 = pse.tile([128, IC * 4], F32, tag="psd", name="psD2")
            psRT = pse.tile([3, SH], F16, tag="psrt", name="psRT")
            n_warm = 0

            def warm(n):
                nonlocal n_warm
                for _ in range(n):
                    pw = pse.tile([128, 512], F32, tag="warm",
                                  name=f"warm{n_warm}")
                    nc.tensor.matmul(
                        pw[:], scr[:, 0:128], scr[:], start=True, stop=True
                    )
                    n_warm += 1

            # psD2[p_i, ic*4+k] = sum_j A[i,j] W'[j,k]; k=3 gives rowsum.
            # One accumulation super-group: start only on the very first
            # matmul (pending-zero covers the whole bank region). Denominator
            # matmuls stream with the at8 pieces; warm-up matmuls fill the
            # PE gaps between pieces so the clock ramp isn't reset.
            warm(4)
            first = True
            for pi, (j0, j1) in enumerate(bounds):
                for jc in range(j0, j1):
                    for ic in range(IC):
                        nc.tensor.matmul(
                            psD2[:, ic * 4:(ic + 1) * 4],
                            at8[:, jc * SH + ic * 128:
                                jc * SH + ic * 128 + 128],
                            w4[:, jc * 4:(jc + 1) * 4],
                            start=first,
                            stop=(jc == JC - 1 and ic == IC - 1),
                            skip_group_check=True,
                        )
                        first = False
                if pi < len(bounds) - 2:
                    warm(3)

            # R' = 1/denom' (fp16), transposed to [k, i]. The rowsum factor
            # is folded into the final psO stores (per-partition scale).
            psD2_v = psD2[:].rearrange("p (ic s) -> p ic s", s=4)
            with nc.allow_low_precision(reason="R' fits fp16 comfortably"):
                nc.vector.reciprocal(
                    rN16[:].rearrange("p (ic k) -> p ic k", k=H),
                    psD2_v[:, :, 0:H],
                )
            nc.vector.tensor_copy(rs32[:], psD2_v[:, :, 3])
            for ic in range(IC):
                nc.tensor.transpose(
                    psRT[:, ic * 128:(ic + 1) * 128],
                    rN16[:, ic * H:(ic + 1) * H],
                    id16[:],
                )
            nc.vector.tensor_copy(rT16[:], psRT[:])

        with (
            tc.tile_pool(name="psc", bufs=2, space="PSUM") as psc,
            tc.tile_pool(name="pso", bufs=1, space="PSUM") as pso,
        ):
            psO = [
                pso.tile([128, D], F32, tag=f"psO{ic}", name=f"psO{ic}")
                for ic in range(IC)
            ]
            hh8_v = hh8[:].rearrange("p (pr two d) -> p pr two d", two=2, d=D)
            hl8_v = hl8[:].rearrange("p (pr two d) -> p pr two d", two=2, d=D)

            # engine assignment per pair for the hi-copy and lo-sub: POOL
            # takes most subs (DVE is mask-bound), but the tail pairs use
            # DVE so the drain chain isn't gated by POOL's 2.1us subs.
            cp_eng = ["act"] * NP
            sub_eng = [
                "dve" if (k % 5 == 0 or k >= NP - 3) else "pool"
                for k in range(NP)
            ]

            his = {}
            los = {}

            def front(k):
                # ct pair -> mask -> hi copy -> lo sub for pair k
                ctp = psc.tile([128, 2 * SH], F32, tag="ctp", name=f"ctp{k}")
                for half in range(2):
                    jc = 2 * k + half
                    nc.tensor.matmul(
                        ctp[:, half * SH:(half + 1) * SH],
                        wt[0:3, jc * 128:(jc + 1) * 128],
                        rT16[:],
                        start=True, stop=True,
                        tile_position=(0, 0),
                    )
                mt16 = mtp.tile([128, 2 * SH], F16, tag="mt16",
                                name=f"mt16_{k}")
                nc.vector.tensor_tensor(
                    mt16[:], at8[:, 2 * k * SH:(2 * k + 2) * SH], ctp[:],
                    op=mult,
                )
                hi8 = mtp.tile([128, 2 * SH], F8, tag="hi8", name=f"hi8_{k}")
                lo8 = mtp.tile([128, 2 * SH], F8, tag="lo8", name=f"lo8_{k}")
                if cp_eng[k] == "act":
                    nc.scalar.copy(hi8[:], mt16[:])
                else:
                    nc.gpsimd.tensor_copy(hi8[:], mt16[:])
                if sub_eng[k] == "dve":
                    nc.vector.tensor_tensor(lo8[:], mt16[:], hi8[:], op=subop)
                else:
                    nc.gpsimd.tensor_tensor(lo8[:], mt16[:], hi8[:], op=subop)
                his[k], los[k] = hi8, lo8

            out_r = out.rearrange("(ic p) d -> ic p d", p=128)

            def store(ic):
                # rowsum-scaled PSUM->SBUF copy + out DMA
                ot = osb.tile([128, D], F16, tag="ot", name=f"ot{ic}")
                if ic % 2 == 0:
                    nc.scalar.mul(ot[:], psO[ic][:], rs32[:, ic:ic + 1])
                else:
                    nc.vector.tensor_scalar(
                        ot[:], psO[ic][:], rs32[:, ic:ic + 1], None, op0=mult
                    )
                (nc.sync if ic % 2 == 0 else nc.scalar).dma_start(
                    out=out_r[ic], in_=ot[:]
                )

            TERMS = ((0, 0), (0, 1), (1, 0))  # (hi/lo, hh/hl)

            def agg(k):
                hi8_v = his[k][:].rearrange("p (two i) -> p two i", two=2)
                lo8_v = los[k][:].rearrange("p (two i) -> p two i", two=2)
                lts = (hi8_v, lo8_v)
                rts = (hh8_v, hl8_v)
                if k < NP - 1:
                    order = [(t, ic) for t in range(3) for ic in range(IC)]
                else:
                    # last pair: ic-major so each psO bank finishes (stop)
                    # as early as possible, with its store right behind
                    order = [(t, ic) for ic in range(IC) for t in range(3)]
                for t, ic in order:
                    lt, rt = lts[TERMS[t][0]], rts[TERMS[t][1]]
                    nc.tensor.matmul(
                        psO[ic][:],
                        lt[:, :, ic * 128:(ic + 1) * 128],
                        rt[:, k],
                        start=(k == 0 and t == 0),
                        stop=(k == NP - 1 and t == 2),
                        perf_mode=DR,
                        skip_group_check=True,
                    )
                    if k == NP - 1 and t == 2:
                        store(ic)

            # software pipeline: the front chain (PE ct -> DVE mask -> ACT
            # copy -> DVE/POOL sub) runs LAG pairs ahead of the aggregation
            # so the per-pair cross-engine latency never stalls the PE.
            LAG = 4
            for k in range(NP + LAG):
                if k < NP:
                    front(k)
                if k >= LAG:
                    agg(k - LAG)


_CACHE = {}


def _build1():
    if "p1" in _CACHE:
        return _CACHE["p1"]
    nc = bacc.Bacc("TRN2", target_bir_lowering=False, debug=False,
                   num_devices=NCORES)
    hst_in = nc.dram_tensor("hst_in", [128, IC * DC * 128], F16,
                            kind="ExternalInput").ap()
    p_in = nc.dram_tensor("p_in", [128, DC * H], F16, kind="ExternalInput").ap()
    w_out = nc.dram_tensor("w_out", [128, IC * H], F16,
                           kind="ExternalOutput").ap()
    with tile.TileContext(nc) as tc:
        _body1(tc, hst_in, p_in, w_out)
    nc.compile()
    _CACHE["p1"] = nc
    return nc


def _build2():
    if "p2" in _CACHE:
        return _CACHE["p2"]
    nc = bacc.Bacc("TRN2", target_bir_lowering=False, debug=False,
                   num_devices=NCORES)
    a8_in = nc.dram_tensor("a8_in", [128, JC * SH], F8,
                           kind="ExternalInput").ap()
    hh_in = nc.dram_tensor("hh_in", [128, JC * D], F8,
                           kind="ExternalInput").ap()
    hl_in = nc.dram_tensor("hl_in", [128, JC * D], F8,
                           kind="ExternalInput").ap()
    wt_in = nc.dram_tensor("wt_in", [3, N], F16, kind="ExternalInput").ap()
    w4_in = nc.dram_tensor("w4_in", [128, JC * 4], F16,
                           kind="ExternalInput").ap()
    id_in = nc.dram_tensor("id_in", [128, 128], F16, kind="ExternalInput").ap()
    out = nc.dram_tensor("out", [SH, D], F16, kind="ExternalOutput").ap()
    with tile.TileContext(nc) as tc:
        _body2(tc, a8_in, hh_in, hl_in, wt_in, w4_in, id_in, out)
    nc.compile()
    _CACHE["p2"] = nc
    return nc


def kernel(graph_info, h, P, _trace=False, _results_out=None):
    graph_info = np.ascontiguousarray(graph_info, dtype=np.float32)
    h = np.ascontiguousarray(h, dtype=np.float32)
    P = np.ascontiguousarray(P, dtype=np.float32)
    nc1 = _build1()
    nc2 = _build2()

    # host-side shard/layout prep (pure data movement + dtype casts)
    h16_full = h.astype(np.float16)
    p16_host = np.ascontiguousarray(
        P.astype(np.float16).reshape(DC, 128, H).transpose(1, 0, 2)
    ).reshape(128, DC * H)
    in1 = []
    for c in range(NCORES):
        hsT = h16_full[c * SH:(c + 1) * SH, :].T  # [D, SH]
        hst_host = np.ascontiguousarray(
            hsT.reshape(DC, 128, IC, 128).transpose(1, 2, 0, 3)
        ).reshape(128, IC * DC * 128)
        in1.append({"hst_in": hst_host, "p_in": p16_host})
    res1 = bass_utils.run_bass_kernel_spmd(
        nc1, in1, core_ids=list(range(NCORES)), trace=_trace
    )
    w_full = np.concatenate(
        [
            res1.results[c]["w_out"]
            .reshape(128, IC, H).transpose(1, 0, 2).reshape(SH, H)
            for c in range(NCORES)
        ],
        axis=0,
    )  # [N, 3] fp16, scaled by 2^-4

    wt_host = np.ascontiguousarray(w_full.T)  # [3, N]
    w4_host = np.ascontiguousarray(
        np.concatenate(
            [w_full.reshape(JC, 128, H).transpose(1, 0, 2),
             np.ones((128, JC, 1), np.float16)],
            axis=2,
        ).reshape(128, JC * 4)
    )
    id_host = np.eye(128, dtype=np.float16)

    # fp8 hi/lo split of h (host-side re-encoding; h = hh + hl up to e4m3^2)
    h_hi = np.clip(h, -240, 240).astype(NP_F8)
    h_lo = (h - h_hi.astype(np.float32)).astype(NP_F8)
    hh_host = np.ascontiguousarray(
        h_hi.reshape(JC, 128, D).transpose(1, 0, 2)).reshape(128, JC * D)
    hl_host = np.ascontiguousarray(
        h_lo.reshape(JC, 128, D).transpose(1, 0, 2)).reshape(128, JC * D)

    in2 = []
    for c in range(NCORES):
        at = np.ascontiguousarray(
            graph_info[c * SH:(c + 1) * SH, :].T
        ).astype(NP_F8)                      # [N(j), SH(i)]
        a8_host = np.ascontiguousarray(
            at.reshape(JC, 128, SH).transpose(1, 0, 2)).reshape(128, JC * SH)
        in2.append({
            "a8_in": a8_host,
            "hh_in": hh_host,
            "hl_in": hl_host,
            "wt_in": wt_host,
            "w4_in": w4_host,
            "id_in": id_host,
        })
    res2 = bass_utils.run_bass_kernel_spmd(
        nc2, in2, core_ids=list(range(NCORES)), trace=_trace
    )
    if _results_out is not None:
        _results_out.extend([res1, res2])
    return np.concatenate(
        [res2.results[c]["out"].astype(np.float32) for c in range(NCORES)],
        axis=0,
    )
